# revision 26
# baseline (speedup 1.0000x reference)
# Multi-head self-attention (B=2, S=4096, D=512, H=8) on 8 NeuronCores.
#
# Sharding: core c -> batch b = c//4, head-pair hp = c%4 (heads 2hp, 2hp+1,
# i.e. channels [128*hp, 128*hp+128) of the QKV projection space).
# Host pre-slices/transposes weights + x per core (cast fp16 for the PE);
# device does all matmuls (QKV projections, flash-attention with fused
# softmax, W_O row-slice projection); host sums the 4 per-core W_O partials
# per batch (the "all-reduce") and transposes back.
#
# Per-core device kernel (matmul operands fp16, accumulation fp32 PSUM):
#   qtz_h/ktz_h [128, 4096]: Q^T/K^T per head, dk on a 64-partition band,
#     zero elsewhere -> every attention matmul is full-K (128,128) mode.
#   scoresT[kpos, q] = K Q^T chunkwise -> ACT exp(x/8) straight from PSUM
#   AV with a ones-column appended to V -> denominator for free
#   softmax division off the critical path (DVE recip + gpsimd broadcast).
# All pools stay open for the whole kernel; PSUM slots are shared between
# phases via tags (8 banks total) so phases overlap with per-slot WAR deps
# instead of pool-close barriers.

import numpy as np

B, S, D, H, DK = 2, 4096, 512, 8, 64
# Schraudolph fp16 exp: bits = round(A*s + B) read as fp16 ~= exp(s/8);
# 0.0430357 centers the multiplicative error of the linear-mantissa
# approximation at +-3%.
EXP_A = 1024.0 * 1.4426950408889634 / 8.0
EXP_B = 15360.0 - 1024.0 * 0.04303566
P = 128          # partition tile
NQ = 512         # matmul moving free dim (one fp32 PSUM bank)
QCH = 1024       # q-chunk (2 x NQ) => one [128,1024] exp per kpos-chunk
NKC = S // P     # kpos chunks (32)
NST = S // NQ    # s-tiles of 512 (8)
NDC = D // P     # d chunks (4)
NQC = S // QCH   # q chunks (4)

TRACE = False            # test.py sets True to get exec_time_ns + perfetto
TMPDIR = None            # optional trace output dir
LAST_RESULTS = None      # BassKernelResults of the last run (for test.py)

_CACHE = {}


def _build_nc():
    import concourse.bass as bass  # noqa: F401
    import concourse.mybir as mybir
    import concourse.tile as tile
    from concourse import bacc
    from concourse.masks import make_identity

    f32 = mybir.dt.float32
    f16 = mybir.dt.float16
    i16 = mybir.dt.int16
    Act = mybir.ActivationFunctionType
    Alu = mybir.AluOpType

    nc = bacc.Bacc("TRN2", target_bir_lowering=False, debug=False, num_devices=8)

    xT = nc.dram_tensor("xT", [D, S], f16, kind="ExternalInput")
    wqT = nc.dram_tensor("wqT", [D, P], f16, kind="ExternalInput")
    wkT = nc.dram_tensor("wkT", [D, P], f16, kind="ExternalInput")
    wvT = nc.dram_tensor("wvT", [D, P], f16, kind="ExternalInput")
    woT0 = nc.dram_tensor("woT0", [DK, D], f16, kind="ExternalInput")
    woT1 = nc.dram_tensor("woT1", [DK, D], f16, kind="ExternalInput")
    yT = nc.dram_tensor("yT", [D, S], f32, kind="ExternalOutput")

    with tile.TileContext(nc) as tc:
        with (
            tc.tile_pool(name="sb", bufs=1) as sb,
            tc.tile_pool(name="ps", bufs=1, space="PSUM") as psp,
        ):
            # PSUM budget (8 banks total, slots shared across phases by tag):
            #   sc0, sc1: [128,1024] -> 2 banks each (scores / exp staging)
            #   av00..av11: [128,512] -> 1 bank each (AV accum; also used by
            #   the QKV-projection psum tiles and the W_O psum tiles)
            def av_ps(i, shape):
                return psp.tile(shape, f32, tag=f"av{i % 4}", name=f"avps{i}")

            # ---- persistent operand tiles -----------------------------------
            qtz = [sb.tile([P, S], f16, tag=f"qtz{h}", name=f"qtz{h}")
                   for h in range(2)]
            ktz = [sb.tile([P, S], f16, tag=f"ktz{h}", name=f"ktz{h}")
                   for h in range(2)]
            vb = [sb.tile([P, NKC * (DK + 1)], f16, tag=f"vb{h}", name=f"vb{h}")
                  for h in range(2)]
            outtz = [sb.tile([P, S], f16, tag=f"outtz{h}", name=f"outtz{h}")
                     for h in range(2)]
            wosz = [sb.tile([P, D], f16, tag=f"wosz{h}", name=f"wosz{h}")
                    for h in range(2)]

            # zero bands + ones-fill on the (otherwise idle) gpsimd engine;
            # bands first: the first scores matmul needs them, vb is needed
            # slightly later by the first AV matmul
            nc.gpsimd.memset(qtz[0][DK:P, :], 0.0)
            nc.gpsimd.memset(ktz[0][DK:P, :], 0.0)
            nc.gpsimd.memset(qtz[1][0:DK, :], 0.0)
            nc.gpsimd.memset(ktz[1][0:DK, :], 0.0)
            nc.gpsimd.memset(vb[0][:, :], 1.0)
            nc.gpsimd.memset(vb[1][:, :], 1.0)
            nc.gpsimd.memset(outtz[0][DK:P, :], 0.0)
            nc.gpsimd.memset(outtz[1][DK:P, :], 0.0)
            nc.gpsimd.memset(wosz[0][DK:P, :], 0.0)
            nc.gpsimd.memset(wosz[1][DK:P, :], 0.0)

            # ---- phase 1: load x + weights, QKV projections, build V -------
            xts = [sb.tile([P, S], f16, tag=f"xt{dc}", name=f"xt{dc}")
                   for dc in range(NDC)]
            wsb = {}
            for name, dram in (("v", wvT), ("k", wkT), ("q", wqT)):
                w = sb.tile([P, NDC * P], f16, tag=f"w{name}", name=f"w{name}")
                for dc in range(NDC):
                    nc.sync.dma_start(
                        w[:, dc * P:(dc + 1) * P], dram[dc * P:(dc + 1) * P, :]
                    )
                wsb[name] = w
            nc.sync.dma_start(wosz[0][0:DK, :], woT0[:, :])
            nc.sync.dma_start(wosz[1][0:DK, :], woT1[:, :])
            for blk in range(8):
                sl = slice(blk * NQ, (blk + 1) * NQ)
                for dc in range(NDC):
                    nc.sync.dma_start(xts[dc][:, sl], xT[dc * P:(dc + 1) * P, sl])

            vt = sb.tile([P, S], f32, tag="vt")
            ident = sb.tile([P, P], f32, tag="ident")
            make_identity(nc, ident[:, :])

            psn = 0
            scn = [0]

            def sc_ps(shape):
                # phase-1 tiles interleaved into the attention stream use the
                # sc tag rotation (1 bank each); the av tags hold live AV
                # accumulators during qc0 and would deadlock
                t = psp.tile(shape, f32, tag=f"sc{scn[0] // 2}{scn[0] % 2}",
                             name=f"p1ps{scn[0]}")
                scn[0] = (scn[0] + 1) % 4
                return t

            def emit_proj(st, names=("v", "k", "q"), late=False):
                for name in names:
                    w = wsb[name]
                    ps = sc_ps([P, NQ]) if late else None
                    if ps is None:
                        nonlocal_psn = emit_proj.psn
                        ps = av_ps(nonlocal_psn, [P, NQ])
                        emit_proj.psn += 1
                    for dc in range(NDC):
                        nc.tensor.matmul(
                            ps[:, :],
                            w[:, dc * P:(dc + 1) * P],
                            xts[dc][:, st * NQ:(st + 1) * NQ],
                            start=(dc == 0),
                            stop=(dc == NDC - 1),
                        )
                    sl = slice(st * NQ, (st + 1) * NQ)
                    if name == "v":
                        nc.vector.tensor_copy(vt[:, sl], ps[:, :])
                        # transpose this V window into per-head V chunks
                        # [kpos, dk] (+ones col at 64) right away
                        for ch in range(4 * st, 4 * st + 4):
                            if late:
                                tp = sc_ps([P, P])
                            else:
                                tp = av_ps(emit_proj.psn, [P, P])
                                emit_proj.psn += 1
                            nc.tensor.transpose(
                                tp[:, :], vt[:, ch * P:(ch + 1) * P], ident[:, :]
                            )
                            c0 = ch * (DK + 1)
                            nc.scalar.copy(vb[0][:, c0:c0 + DK], tp[:, 0:DK])
                            nc.vector.tensor_copy(
                                vb[1][:, c0:c0 + DK], tp[:, DK:P]
                            )
                    elif name == "k":
                        nc.scalar.copy(ktz[0][0:DK, sl], ps[0:DK, :])
                        nc.scalar.copy(ktz[1][DK:P, sl], ps[DK:P, :])
                    else:
                        nc.vector.tensor_copy(qtz[0][0:DK, sl], ps[0:DK, :])
                        nc.vector.tensor_copy(qtz[1][DK:P, sl], ps[DK:P, :])

            emit_proj.psn = 0
            # upfront: everything attention chunks 0..3 need (x blks 0-1 of
            # DMA); later s-tiles interleave into qc0's chunk loop so the PE
            # computes attention while the rest of x streams in
            emit_proj(0, names=("q", "v", "k"))
            emit_proj(1, names=("q",))
            psn = emit_proj.psn

            # ---- phase 2: flash attention -----------------------------------
            def emit_normalize(qc, av):
                # evacuate av psum fast (frees the bank), then the softmax
                # division off the critical path in SBUF
                for h in range(2):
                    for sub in range(2):
                        a = av[h, sub]
                        raw = sb.tile([DK + 1, NQ], f32, tag=f"raw{h}{sub}",
                                      name=f"raw{h}{sub}", bufs=3)
                        nc.vector.tensor_copy(raw[:, :], a[0:DK + 1, :])
                        dn0 = sb.tile([P, NQ], f32, tag="dn0", bufs=4)
                        nc.sync.dma_start(dn0[0:1, :], raw[DK:DK + 1, :])
                        rc = sb.tile([P, NQ], f32, tag="rc", bufs=4)
                        nc.vector.reciprocal_approx_fast(rc[0:1, :], dn0[0:1, :])
                        rcb = sb.tile([DK, NQ], f32, tag="rcb", bufs=4)
                        nc.gpsimd.partition_broadcast(
                            rcb[:, :], rc[0:1, :], channels=DK
                        )
                        q0 = qc * QCH + sub * NQ
                        nc.vector.tensor_mul(
                            outtz[h][0:DK, q0:q0 + NQ], raw[0:DK, :], rcb[:, :]
                        )

            # W_O for one q-chunk's two s-tiles -- emitted right after
            # that chunk's normalize so the projection overlaps the next
            # chunk's attention instead of tailing the kernel
            def emit_wo(qc):
                nonlocal psn
                for st in (2 * qc, 2 * qc + 1):
                    for ec in range(NDC):
                        yp = av_ps(psn, [P, NQ])
                        psn += 1
                        for h in range(2):
                            nc.tensor.matmul(
                                yp[:, :],
                                wosz[h][:, ec * P:(ec + 1) * P],
                                outtz[h][:, st * NQ:(st + 1) * NQ],
                                start=(h == 0),
                                stop=(h == 1),
                            )
                        ys = sb.tile([P, NQ], f32, tag="ys", bufs=4)
                        nc.scalar.copy(ys[:, :], yp[:, :])
                        nc.sync.dma_start(
                            yT[ec * P:(ec + 1) * P, st * NQ:(st + 1) * NQ],
                            ys[:, :],
                        )

            pending = None
            for qc in range(NQC):
                av = {}
                for h in range(2):
                    for sub in range(2):
                        av[h, sub] = av_ps(psn, [P, NQ])
                        psn += 1
                for k in range(NKC):
                    if qc == 0 and 1 <= k <= 7 and k % 1 == 0 and False:
                        pass
                    if qc == 0 and k in (1, 2, 3, 4, 5, 6, 7):
                        st = k
                        emit_proj(st, names=("v", "k"), late=True)
                        if st + 1 < NST:
                            emit_proj(st + 1, names=("q",), late=True)
                    if k == 3 and pending is not None:
                        emit_normalize(*pending)
                        emit_wo(pending[0])
                        pending = None
                    # one single-bank psum tile per scores matmul, so each
                    # exp sub-op depends on exactly one matmul and each WAR
                    # on exactly one exp op (no 2-bank tile-level coupling)
                    scps = [[psp.tile([P, NQ], f32, tag=f"sc{h}{sub}",
                                      name=f"sc{h}{sub}")
                             for sub in range(2)] for h in range(2)]
                    for h in range(2):
                        for sub in range(2):
                            q0 = qc * QCH + sub * NQ
                            nc.tensor.matmul(
                                scps[h][sub][:, :],
                                ktz[h][:, k * P:(k + 1) * P],
                                qtz[h][:, q0:q0 + NQ],
                                start=True,
                                stop=True,
                            )
                    # exp: head0 on ACT (true exp), head1 on the DVE as a
                    # Schraudolph bit-trick exp -- int16(A*s+B) whose bit
                    # pattern read as fp16 is exp(s/8) within ~+-3%
                    # (mean-free; the softmax ratio cancels most of it)
                    ex0 = sb.tile([P, QCH], f16, tag="ex0", name="ex0", bufs=6)
                    ex1i = sb.tile([P, QCH], i16, tag="ex1", name="ex1", bufs=6)
                    for sub in range(2):
                        ssl = slice(sub * NQ, (sub + 1) * NQ)
                        nc.scalar.activation(
                            ex0[:, ssl], scps[0][sub][:, :], Act.Exp,
                            scale=0.125
                        )
                        nc.vector.tensor_scalar(
                            ex1i[:, ssl], scps[1][sub][:, :], EXP_A, EXP_B,
                            Alu.mult, Alu.add,
                        )
                    exs = [ex0, ex1i.bitcast(f16)]
                    c0 = k * (DK + 1)
                    for h in range(2):
                        for sub in range(2):
                            nc.tensor.matmul(
                                av[h, sub][0:DK + 1, :],
                                vb[h][:, c0:c0 + DK + 1],
                                exs[h][:, sub * NQ:(sub + 1) * NQ],
                                start=(k == 0),
                                stop=(k == NKC - 1),
                            )
                pending = (qc, av)
            emit_normalize(*pending)
            emit_wo(pending[0])

    nc.compile()
    return nc


def kernel(x, wq, wk, wv, wo):
    global LAST_RESULTS
    from concourse.bass_utils import run_bass_kernel_spmd

    if "nc" not in _CACHE:
        _CACHE["nc"] = _build_nc()
    nc = _CACHE["nc"]

    x = np.asarray(x, dtype=np.float32)
    wq = np.asarray(wq, dtype=np.float32)
    wk = np.asarray(wk, dtype=np.float32)
    wv = np.asarray(wv, dtype=np.float32)
    wo = np.asarray(wo, dtype=np.float32)

    in_maps = []
    for c in range(8):
        b, hp = divmod(c, 4)
        e0 = hp * P
        in_maps.append({
            "xT": np.ascontiguousarray(x[b].T.astype(np.float16)),
            "wqT": np.ascontiguousarray(wq[e0:e0 + P].T.astype(np.float16)),
            "wkT": np.ascontiguousarray(wk[e0:e0 + P].T.astype(np.float16)),
            "wvT": np.ascontiguousarray(wv[e0:e0 + P].T.astype(np.float16)),
            "woT0": np.ascontiguousarray(wo[:, e0:e0 + DK].T.astype(np.float16)),
            "woT1": np.ascontiguousarray(wo[:, e0 + DK:e0 + P].T.astype(np.float16)),
        })

    res = run_bass_kernel_spmd(
        nc, in_maps, core_ids=list(range(8)), trace=TRACE, tmpdir=TMPDIR
    )
    LAST_RESULTS = res

    y = np.zeros((B, S, D), dtype=np.float32)
    for c in range(8):
        y[c // 4] += res.results[c]["yT"].T
    return y



# revision 28
# speedup vs baseline: 1.0065x; 1.0065x over previous
# Multi-head self-attention (B=2, S=4096, D=512, H=8) on 8 NeuronCores.
#
# Sharding: core c -> batch b = c//4, head-pair hp = c%4 (heads 2hp, 2hp+1,
# i.e. channels [128*hp, 128*hp+128) of the QKV projection space).
# Host pre-slices/transposes weights + x per core (cast fp16 for the PE);
# device does all matmuls (QKV projections, flash-attention with fused
# softmax, W_O row-slice projection); host sums the 4 per-core W_O partials
# per batch (the "all-reduce") and transposes back.
#
# Per-core device kernel (matmul operands fp16, accumulation fp32 PSUM):
#   qtz_h/ktz_h [128, 4096]: Q^T/K^T per head, dk on a 64-partition band,
#     zero elsewhere -> every attention matmul is full-K (128,128) mode.
#   scoresT[kpos, q] = K Q^T chunkwise -> ACT exp(x/8) straight from PSUM
#   AV with a ones-column appended to V -> denominator for free
#   softmax division off the critical path (DVE recip + gpsimd broadcast).
# All pools stay open for the whole kernel; PSUM slots are shared between
# phases via tags (8 banks total) so phases overlap with per-slot WAR deps
# instead of pool-close barriers.

import numpy as np

B, S, D, H, DK = 2, 4096, 512, 8, 64
# Schraudolph fp16 exp: bits = round(A*s + B) read as fp16 ~= exp(s/8);
# 0.0430357 centers the multiplicative error of the linear-mantissa
# approximation at +-3%.
EXP_A = 1024.0 * 1.4426950408889634 / 8.0
EXP_B = 15360.0 - 1024.0 * 0.04303566
P = 128          # partition tile
NQ = 512         # matmul moving free dim (one fp32 PSUM bank)
QCH = 1024       # q-chunk (2 x NQ) => one [128,1024] exp per kpos-chunk
NKC = S // P     # kpos chunks (32)
NST = S // NQ    # s-tiles of 512 (8)
NDC = D // P     # d chunks (4)
NQC = S // QCH   # q chunks (4)

TRACE = False            # test.py sets True to get exec_time_ns + perfetto
TMPDIR = None            # optional trace output dir
LAST_RESULTS = None      # BassKernelResults of the last run (for test.py)

_CACHE = {}


def _build_nc():
    import concourse.bass as bass  # noqa: F401
    import concourse.mybir as mybir
    import concourse.tile as tile
    from concourse import bacc
    from concourse.masks import make_identity

    f32 = mybir.dt.float32
    f16 = mybir.dt.float16
    i16 = mybir.dt.int16
    Act = mybir.ActivationFunctionType
    Alu = mybir.AluOpType

    nc = bacc.Bacc("TRN2", target_bir_lowering=False, debug=False, num_devices=8)

    xT = nc.dram_tensor("xT", [D, S], f16, kind="ExternalInput")
    wqT = nc.dram_tensor("wqT", [D, P], f16, kind="ExternalInput")
    wkT = nc.dram_tensor("wkT", [D, P], f16, kind="ExternalInput")
    wvT = nc.dram_tensor("wvT", [D, P], f16, kind="ExternalInput")
    woT0 = nc.dram_tensor("woT0", [DK, D], f16, kind="ExternalInput")
    woT1 = nc.dram_tensor("woT1", [DK, D], f16, kind="ExternalInput")
    yT = nc.dram_tensor("yT", [D, S], f32, kind="ExternalOutput")

    with tile.TileContext(nc) as tc:
        with (
            tc.tile_pool(name="sb", bufs=1) as sb,
            tc.tile_pool(name="ps", bufs=1, space="PSUM") as psp,
        ):
            # PSUM budget (8 banks total, slots shared across phases by tag):
            #   sc0, sc1: [128,1024] -> 2 banks each (scores / exp staging)
            #   av00..av11: [128,512] -> 1 bank each (AV accum; also used by
            #   the QKV-projection psum tiles and the W_O psum tiles)
            def av_ps(i, shape):
                return psp.tile(shape, f32, tag=f"av{i % 4}", name=f"avps{i}")

            # ---- persistent operand tiles -----------------------------------
            qtz = [sb.tile([P, S], f16, tag=f"qtz{h}", name=f"qtz{h}")
                   for h in range(2)]
            ktz = [sb.tile([P, S], f16, tag=f"ktz{h}", name=f"ktz{h}")
                   for h in range(2)]
            vb = [sb.tile([P, NKC * (DK + 1)], f16, tag=f"vb{h}", name=f"vb{h}")
                  for h in range(2)]
            outtz = [sb.tile([P, S], f16, tag=f"outtz{h}", name=f"outtz{h}")
                     for h in range(2)]
            wosz = [sb.tile([P, D], f16, tag=f"wosz{h}", name=f"wosz{h}")
                    for h in range(2)]

            # zero bands + ones-fill on the (otherwise idle) gpsimd engine;
            # bands first: the first scores matmul needs them, vb is needed
            # slightly later by the first AV matmul
            nc.gpsimd.memset(qtz[0][DK:P, :], 0.0)
            nc.gpsimd.memset(ktz[0][DK:P, :], 0.0)
            nc.gpsimd.memset(qtz[1][0:DK, :], 0.0)
            nc.gpsimd.memset(ktz[1][0:DK, :], 0.0)
            nc.gpsimd.memset(vb[0][:, :], 1.0)
            nc.gpsimd.memset(vb[1][:, :], 1.0)
            nc.gpsimd.memset(outtz[0][DK:P, :], 0.0)
            nc.gpsimd.memset(outtz[1][DK:P, :], 0.0)
            nc.gpsimd.memset(wosz[0][DK:P, :], 0.0)
            nc.gpsimd.memset(wosz[1][DK:P, :], 0.0)

            # ---- phase 1: load x + weights, QKV projections, build V -------
            xts = [sb.tile([P, S], f16, tag=f"xt{dc}", name=f"xt{dc}")
                   for dc in range(NDC)]
            wsb = {}
            for name, dram in (("v", wvT), ("k", wkT), ("q", wqT)):
                w = sb.tile([P, NDC * P], f16, tag=f"w{name}", name=f"w{name}")
                for dc in range(NDC):
                    nc.sync.dma_start(
                        w[:, dc * P:(dc + 1) * P], dram[dc * P:(dc + 1) * P, :]
                    )
                wsb[name] = w
            nc.sync.dma_start(wosz[0][0:DK, :], woT0[:, :])
            nc.sync.dma_start(wosz[1][0:DK, :], woT1[:, :])
            for blk in range(8):
                sl = slice(blk * NQ, (blk + 1) * NQ)
                for dc in range(NDC):
                    nc.sync.dma_start(xts[dc][:, sl], xT[dc * P:(dc + 1) * P, sl])

            vt = sb.tile([P, S], f32, tag="vt")
            ident = sb.tile([P, P], f32, tag="ident")
            make_identity(nc, ident[:, :])

            psn = 0
            for st in range(NST):
                for name in ("v", "k", "q"):
                    w = wsb[name]
                    ps = av_ps(psn, [P, NQ])
                    psn += 1
                    for dc in range(NDC):
                        nc.tensor.matmul(
                            ps[:, :],
                            w[:, dc * P:(dc + 1) * P],
                            xts[dc][:, st * NQ:(st + 1) * NQ],
                            start=(dc == 0),
                            stop=(dc == NDC - 1),
                        )
                    sl = slice(st * NQ, (st + 1) * NQ)
                    if name == "v":
                        nc.vector.tensor_copy(vt[:, sl], ps[:, :])
                        # transpose this V window into per-head V chunks
                        # [kpos, dk] (+ones col at 64) right away
                        for ch in range(4 * st, 4 * st + 4):
                            tp = av_ps(psn, [P, P])
                            psn += 1
                            nc.tensor.transpose(
                                tp[:, :], vt[:, ch * P:(ch + 1) * P], ident[:, :]
                            )
                            c0 = ch * (DK + 1)
                            nc.scalar.copy(vb[0][:, c0:c0 + DK], tp[:, 0:DK])
                            nc.vector.tensor_copy(
                                vb[1][:, c0:c0 + DK], tp[:, DK:P]
                            )
                    elif name == "k":
                        nc.scalar.copy(ktz[0][0:DK, sl], ps[0:DK, :])
                        nc.scalar.copy(ktz[1][DK:P, sl], ps[DK:P, :])
                    else:
                        nc.vector.tensor_copy(qtz[0][0:DK, sl], ps[0:DK, :])
                        nc.vector.tensor_copy(qtz[1][DK:P, sl], ps[DK:P, :])

            # ---- phase 2: flash attention -----------------------------------
            def emit_normalize(qc, av):
                # evacuate av psum fast (frees the bank), then the softmax
                # division off the critical path in SBUF
                for h in range(2):
                    for sub in range(2):
                        a = av[h, sub]
                        raw = sb.tile([DK + 1, NQ], f32, tag=f"raw{h}{sub}",
                                      name=f"raw{h}{sub}", bufs=3)
                        nc.vector.tensor_copy(raw[:, :], a[0:DK + 1, :])
                        dn0 = sb.tile([P, NQ], f32, tag="dn0", bufs=4)
                        nc.sync.dma_start(dn0[0:1, :], raw[DK:DK + 1, :])
                        rc = sb.tile([P, NQ], f32, tag="rc", bufs=4)
                        nc.vector.reciprocal_approx_fast(rc[0:1, :], dn0[0:1, :])
                        rcb = sb.tile([DK, NQ], f32, tag="rcb", bufs=4)
                        nc.gpsimd.partition_broadcast(
                            rcb[:, :], rc[0:1, :], channels=DK
                        )
                        q0 = qc * QCH + sub * NQ
                        nc.vector.tensor_mul(
                            outtz[h][0:DK, q0:q0 + NQ], raw[0:DK, :], rcb[:, :]
                        )

            # W_O for one q-chunk's two s-tiles -- emitted right after
            # that chunk's normalize so the projection overlaps the next
            # chunk's attention instead of tailing the kernel
            def emit_wo(qc):
                nonlocal psn
                for st in (2 * qc, 2 * qc + 1):
                    for ec in range(NDC):
                        yp = av_ps(psn, [P, NQ])
                        psn += 1
                        for h in range(2):
                            nc.tensor.matmul(
                                yp[:, :],
                                wosz[h][:, ec * P:(ec + 1) * P],
                                outtz[h][:, st * NQ:(st + 1) * NQ],
                                start=(h == 0),
                                stop=(h == 1),
                            )
                        ys = sb.tile([P, NQ], f32, tag="ys", bufs=4)
                        if ec % 2 == 0:
                            nc.scalar.copy(ys[:, :], yp[:, :])
                        else:
                            nc.vector.tensor_copy(ys[:, :], yp[:, :])
                        nc.sync.dma_start(
                            yT[ec * P:(ec + 1) * P, st * NQ:(st + 1) * NQ],
                            ys[:, :],
                        )

            pending = None
            for qc in range(NQC):
                av = {}
                for h in range(2):
                    for sub in range(2):
                        av[h, sub] = av_ps(psn, [P, NQ])
                        psn += 1
                for k in range(NKC):
                    if k == 3 and pending is not None:
                        emit_normalize(*pending)
                        emit_wo(pending[0])
                        pending = None
                    # one single-bank psum tile per scores matmul, so each
                    # exp sub-op depends on exactly one matmul and each WAR
                    # on exactly one exp op (no 2-bank tile-level coupling)
                    scps = [[psp.tile([P, NQ], f32, tag=f"sc{h}{sub}",
                                      name=f"sc{h}{sub}")
                             for sub in range(2)] for h in range(2)]
                    for h in range(2):
                        for sub in range(2):
                            q0 = qc * QCH + sub * NQ
                            nc.tensor.matmul(
                                scps[h][sub][:, :],
                                ktz[h][:, k * P:(k + 1) * P],
                                qtz[h][:, q0:q0 + NQ],
                                start=True,
                                stop=True,
                            )
                    # exp: head0 on ACT (true exp), head1 on the DVE as a
                    # Schraudolph bit-trick exp -- int16(A*s+B) whose bit
                    # pattern read as fp16 is exp(s/8) within ~+-3%
                    # (mean-free; the softmax ratio cancels most of it)
                    ex0 = sb.tile([P, QCH], f16, tag="ex0", name="ex0", bufs=6)
                    ex1i = sb.tile([P, QCH], i16, tag="ex1", name="ex1", bufs=6)
                    for sub in range(2):
                        ssl = slice(sub * NQ, (sub + 1) * NQ)
                        nc.scalar.activation(
                            ex0[:, ssl], scps[0][sub][:, :], Act.Exp,
                            scale=0.125
                        )
                        nc.vector.tensor_scalar(
                            ex1i[:, ssl], scps[1][sub][:, :], EXP_A, EXP_B,
                            Alu.mult, Alu.add,
                        )
                    exs = [ex0, ex1i.bitcast(f16)]
                    c0 = k * (DK + 1)
                    for h in range(2):
                        for sub in range(2):
                            nc.tensor.matmul(
                                av[h, sub][0:DK + 1, :],
                                vb[h][:, c0:c0 + DK + 1],
                                exs[h][:, sub * NQ:(sub + 1) * NQ],
                                start=(k == 0),
                                stop=(k == NKC - 1),
                            )
                pending = (qc, av)
            emit_normalize(*pending)
            emit_wo(pending[0])

    nc.compile()
    return nc


def kernel(x, wq, wk, wv, wo):
    global LAST_RESULTS
    from concourse.bass_utils import run_bass_kernel_spmd

    if "nc" not in _CACHE:
        _CACHE["nc"] = _build_nc()
    nc = _CACHE["nc"]

    x = np.asarray(x, dtype=np.float32)
    wq = np.asarray(wq, dtype=np.float32)
    wk = np.asarray(wk, dtype=np.float32)
    wv = np.asarray(wv, dtype=np.float32)
    wo = np.asarray(wo, dtype=np.float32)

    in_maps = []
    for c in range(8):
        b, hp = divmod(c, 4)
        e0 = hp * P
        in_maps.append({
            "xT": np.ascontiguousarray(x[b].T.astype(np.float16)),
            "wqT": np.ascontiguousarray(wq[e0:e0 + P].T.astype(np.float16)),
            "wkT": np.ascontiguousarray(wk[e0:e0 + P].T.astype(np.float16)),
            "wvT": np.ascontiguousarray(wv[e0:e0 + P].T.astype(np.float16)),
            "woT0": np.ascontiguousarray(wo[:, e0:e0 + DK].T.astype(np.float16)),
            "woT1": np.ascontiguousarray(wo[:, e0 + DK:e0 + P].T.astype(np.float16)),
        })

    res = run_bass_kernel_spmd(
        nc, in_maps, core_ids=list(range(8)), trace=TRACE, tmpdir=TMPDIR
    )
    LAST_RESULTS = res

    y = np.zeros((B, S, D), dtype=np.float32)
    for c in range(8):
        y[c // 4] += res.results[c]["yT"].T
    return y



# revision 30
# speedup vs baseline: 1.0138x; 1.0072x over previous
# Multi-head self-attention (B=2, S=4096, D=512, H=8) on 8 NeuronCores.
#
# Sharding: core c -> batch b = c//4, head-pair hp = c%4 (heads 2hp, 2hp+1,
# i.e. channels [128*hp, 128*hp+128) of the QKV projection space).
# Host pre-slices/transposes weights + x per core (cast fp16 for the PE);
# device does all matmuls (QKV projections, flash-attention with fused
# softmax, W_O row-slice projection); host sums the 4 per-core W_O partials
# per batch (the "all-reduce") and transposes back.
#
# Per-core device kernel (matmul operands fp16, accumulation fp32 PSUM):
#   qtz_h/ktz_h [128, 4096]: Q^T/K^T per head, dk on a 64-partition band,
#     zero elsewhere -> every attention matmul is full-K (128,128) mode.
#   scoresT[kpos, q] = K Q^T chunkwise -> ACT exp(x/8) straight from PSUM
#   AV with a ones-column appended to V -> denominator for free
#   softmax division off the critical path (DVE recip + gpsimd broadcast).
# All pools stay open for the whole kernel; PSUM slots are shared between
# phases via tags (8 banks total) so phases overlap with per-slot WAR deps
# instead of pool-close barriers.

import numpy as np

B, S, D, H, DK = 2, 4096, 512, 8, 64
# Schraudolph fp16 exp: bits = round(A*s + B) read as fp16 ~= exp(s/8);
# 0.0430357 centers the multiplicative error of the linear-mantissa
# approximation at +-3%.
EXP_A = 1024.0 * 1.4426950408889634 / 8.0
EXP_B = 15360.0 - 1024.0 * 0.04303566
P = 128          # partition tile
NQ = 512         # matmul moving free dim (one fp32 PSUM bank)
QCH = 1024       # q-chunk (2 x NQ) => one [128,1024] exp per kpos-chunk
NKC = S // P     # kpos chunks (32)
NST = S // NQ    # s-tiles of 512 (8)
NDC = D // P     # d chunks (4)
NQC = S // QCH   # q chunks (4)

TRACE = False            # test.py sets True to get exec_time_ns + perfetto
TMPDIR = None            # optional trace output dir
LAST_RESULTS = None      # BassKernelResults of the last run (for test.py)

_CACHE = {}


def _build_nc():
    import concourse.bass as bass  # noqa: F401
    import concourse.mybir as mybir
    import concourse.tile as tile
    from concourse import bacc
    from concourse.masks import make_identity

    f32 = mybir.dt.float32
    f16 = mybir.dt.float16
    i16 = mybir.dt.int16
    Act = mybir.ActivationFunctionType
    Alu = mybir.AluOpType

    nc = bacc.Bacc("TRN2", target_bir_lowering=False, debug=False, num_devices=8)

    xT = nc.dram_tensor("xT", [D, S], f16, kind="ExternalInput")
    # weight slices pre-arranged by the host in the exact SBUF layout
    # ([P, D] row-slice of the torch weight) -> one contiguous DMA each
    wqS = nc.dram_tensor("wqS", [P, D], f16, kind="ExternalInput")
    wkS = nc.dram_tensor("wkS", [P, D], f16, kind="ExternalInput")
    wvS = nc.dram_tensor("wvS", [P, D], f16, kind="ExternalInput")
    woT0 = nc.dram_tensor("woT0", [DK, D], f16, kind="ExternalInput")
    woT1 = nc.dram_tensor("woT1", [DK, D], f16, kind="ExternalInput")
    yT = nc.dram_tensor("yT", [D, S], f32, kind="ExternalOutput")

    with tile.TileContext(nc) as tc:
        with (
            tc.tile_pool(name="sb", bufs=1) as sb,
            tc.tile_pool(name="ps", bufs=1, space="PSUM") as psp,
        ):
            # PSUM budget (8 banks total, slots shared across phases by tag):
            #   sc0, sc1: [128,1024] -> 2 banks each (scores / exp staging)
            #   av00..av11: [128,512] -> 1 bank each (AV accum; also used by
            #   the QKV-projection psum tiles and the W_O psum tiles)
            def av_ps(i, shape):
                return psp.tile(shape, f32, tag=f"av{i % 4}", name=f"avps{i}")

            # ---- persistent operand tiles -----------------------------------
            qtz = [sb.tile([P, S], f16, tag=f"qtz{h}", name=f"qtz{h}")
                   for h in range(2)]
            ktz = [sb.tile([P, S], f16, tag=f"ktz{h}", name=f"ktz{h}")
                   for h in range(2)]
            vb = [sb.tile([P, NKC * (DK + 1)], f16, tag=f"vb{h}", name=f"vb{h}")
                  for h in range(2)]
            outtz = [sb.tile([P, S], f16, tag=f"outtz{h}", name=f"outtz{h}")
                     for h in range(2)]
            wosz = [sb.tile([P, D], f16, tag=f"wosz{h}", name=f"wosz{h}")
                    for h in range(2)]

            # zero bands + ones-fill on the (otherwise idle) gpsimd engine;
            # bands first: the first scores matmul needs them, vb is needed
            # slightly later by the first AV matmul
            nc.gpsimd.memset(qtz[0][DK:P, :], 0.0)
            nc.gpsimd.memset(ktz[0][DK:P, :], 0.0)
            nc.gpsimd.memset(qtz[1][0:DK, :], 0.0)
            nc.gpsimd.memset(ktz[1][0:DK, :], 0.0)
            nc.gpsimd.memset(vb[0][:, :], 1.0)
            nc.gpsimd.memset(vb[1][:, :], 1.0)
            nc.gpsimd.memset(outtz[0][DK:P, :], 0.0)
            nc.gpsimd.memset(outtz[1][DK:P, :], 0.0)
            nc.gpsimd.memset(wosz[0][DK:P, :], 0.0)
            nc.gpsimd.memset(wosz[1][DK:P, :], 0.0)

            # ---- phase 1: load x + weights, QKV projections, build V -------
            xts = [sb.tile([P, S], f16, tag=f"xt{dc}", name=f"xt{dc}")
                   for dc in range(NDC)]
            wsb = {}
            for name, dram in (("v", wvS), ("k", wkS), ("q", wqS)):
                w = sb.tile([P, NDC * P], f16, tag=f"w{name}", name=f"w{name}")
                nc.sync.dma_start(w[:, :], dram[:, :])
                wsb[name] = w
            nc.sync.dma_start(wosz[0][0:DK, :], woT0[:, :])
            nc.sync.dma_start(wosz[1][0:DK, :], woT1[:, :])
            # x in two wide transfers per d-chunk: few triggers (the ~0.7us
            # per-descriptor sync cost dominated startup with 32 small ones),
            # while the half split still lets s-tiles 0-3 start early
            for half in range(2):
                hs = slice(half * (S // 2), (half + 1) * (S // 2))
                for dc in range(NDC):
                    nc.sync.dma_start(xts[dc][:, hs], xT[dc * P:(dc + 1) * P, hs])

            vt = sb.tile([P, S], f32, tag="vt")
            ident = sb.tile([P, P], f32, tag="ident")
            make_identity(nc, ident[:, :])

            psn = 0
            for st in range(NST):
                for name in ("v", "k", "q"):
                    w = wsb[name]
                    ps = av_ps(psn, [P, NQ])
                    psn += 1
                    for dc in range(NDC):
                        nc.tensor.matmul(
                            ps[:, :],
                            w[:, dc * P:(dc + 1) * P],
                            xts[dc][:, st * NQ:(st + 1) * NQ],
                            start=(dc == 0),
                            stop=(dc == NDC - 1),
                        )
                    sl = slice(st * NQ, (st + 1) * NQ)
                    if name == "v":
                        nc.vector.tensor_copy(vt[:, sl], ps[:, :])
                        # transpose this V window into per-head V chunks
                        # [kpos, dk] (+ones col at 64) right away
                        for ch in range(4 * st, 4 * st + 4):
                            tp = av_ps(psn, [P, P])
                            psn += 1
                            nc.tensor.transpose(
                                tp[:, :], vt[:, ch * P:(ch + 1) * P], ident[:, :]
                            )
                            c0 = ch * (DK + 1)
                            nc.scalar.copy(vb[0][:, c0:c0 + DK], tp[:, 0:DK])
                            nc.vector.tensor_copy(
                                vb[1][:, c0:c0 + DK], tp[:, DK:P]
                            )
                    elif name == "k":
                        nc.scalar.copy(ktz[0][0:DK, sl], ps[0:DK, :])
                        nc.scalar.copy(ktz[1][DK:P, sl], ps[DK:P, :])
                    else:
                        nc.vector.tensor_copy(qtz[0][0:DK, sl], ps[0:DK, :])
                        nc.vector.tensor_copy(qtz[1][DK:P, sl], ps[DK:P, :])

            # ---- phase 2: flash attention -----------------------------------
            def emit_normalize(qc, av):
                # evacuate av psum fast (frees the bank), then the softmax
                # division off the critical path in SBUF
                for h in range(2):
                    for sub in range(2):
                        a = av[h, sub]
                        raw = sb.tile([DK + 1, NQ], f32, tag=f"raw{h}{sub}",
                                      name=f"raw{h}{sub}", bufs=3)
                        nc.vector.tensor_copy(raw[:, :], a[0:DK + 1, :])
                        dn0 = sb.tile([P, NQ], f32, tag="dn0", bufs=4)
                        nc.sync.dma_start(dn0[0:1, :], raw[DK:DK + 1, :])
                        rc = sb.tile([P, NQ], f32, tag="rc", bufs=4)
                        nc.vector.reciprocal_approx_fast(rc[0:1, :], dn0[0:1, :])
                        rcb = sb.tile([DK, NQ], f32, tag="rcb", bufs=4)
                        nc.gpsimd.partition_broadcast(
                            rcb[:, :], rc[0:1, :], channels=DK
                        )
                        q0 = qc * QCH + sub * NQ
                        nc.vector.tensor_mul(
                            outtz[h][0:DK, q0:q0 + NQ], raw[0:DK, :], rcb[:, :]
                        )

            # W_O for one q-chunk's two s-tiles -- emitted right after
            # that chunk's normalize so the projection overlaps the next
            # chunk's attention instead of tailing the kernel
            def emit_wo(qc):
                nonlocal psn
                for st in (2 * qc, 2 * qc + 1):
                    for ec in range(NDC):
                        yp = av_ps(psn, [P, NQ])
                        psn += 1
                        for h in range(2):
                            nc.tensor.matmul(
                                yp[:, :],
                                wosz[h][:, ec * P:(ec + 1) * P],
                                outtz[h][:, st * NQ:(st + 1) * NQ],
                                start=(h == 0),
                                stop=(h == 1),
                            )
                        ys = sb.tile([P, NQ], f32, tag="ys", bufs=4)
                        nc.scalar.copy(ys[:, :], yp[:, :])
                        nc.sync.dma_start(
                            yT[ec * P:(ec + 1) * P, st * NQ:(st + 1) * NQ],
                            ys[:, :],
                        )

            pending = None
            for qc in range(NQC):
                av = {}
                for h in range(2):
                    for sub in range(2):
                        av[h, sub] = av_ps(psn, [P, NQ])
                        psn += 1
                for k in range(NKC):
                    if k == 3 and pending is not None:
                        emit_normalize(*pending)
                        emit_wo(pending[0])
                        pending = None
                    # one single-bank psum tile per scores matmul, so each
                    # exp sub-op depends on exactly one matmul and each WAR
                    # on exactly one exp op (no 2-bank tile-level coupling)
                    scps = [[psp.tile([P, NQ], f32, tag=f"sc{h}{sub}",
                                      name=f"sc{h}{sub}")
                             for sub in range(2)] for h in range(2)]
                    for h in range(2):
                        for sub in range(2):
                            q0 = qc * QCH + sub * NQ
                            nc.tensor.matmul(
                                scps[h][sub][:, :],
                                ktz[h][:, k * P:(k + 1) * P],
                                qtz[h][:, q0:q0 + NQ],
                                start=True,
                                stop=True,
                            )
                    # exp: head0 on ACT (true exp), head1 on the DVE as a
                    # Schraudolph bit-trick exp -- int16(A*s+B) whose bit
                    # pattern read as fp16 is exp(s/8) within ~+-3%
                    # (mean-free; the softmax ratio cancels most of it)
                    ex0 = sb.tile([P, QCH], f16, tag="ex0", name="ex0", bufs=6)
                    ex1i = sb.tile([P, QCH], i16, tag="ex1", name="ex1", bufs=6)
                    for sub in range(2):
                        ssl = slice(sub * NQ, (sub + 1) * NQ)
                        nc.scalar.activation(
                            ex0[:, ssl], scps[0][sub][:, :], Act.Exp,
                            scale=0.125
                        )
                        nc.vector.tensor_scalar(
                            ex1i[:, ssl], scps[1][sub][:, :], EXP_A, EXP_B,
                            Alu.mult, Alu.add,
                        )
                    exs = [ex0, ex1i.bitcast(f16)]
                    c0 = k * (DK + 1)
                    for h in range(2):
                        for sub in range(2):
                            nc.tensor.matmul(
                                av[h, sub][0:DK + 1, :],
                                vb[h][:, c0:c0 + DK + 1],
                                exs[h][:, sub * NQ:(sub + 1) * NQ],
                                start=(k == 0),
                                stop=(k == NKC - 1),
                            )
                pending = (qc, av)
            emit_normalize(*pending)
            emit_wo(pending[0])

    nc.compile()
    return nc


def _wsb(w, e0):
    # SBUF weight layout: w_sb[p, dc*P + m] = w[e0 + m, dc*P + p]
    wt = w[e0:e0 + P].T.astype(np.float16)          # [D, P]
    return np.ascontiguousarray(
        np.hstack([wt[d * P:(d + 1) * P, :] for d in range(NDC)])
    )


def kernel(x, wq, wk, wv, wo):
    global LAST_RESULTS
    from concourse.bass_utils import run_bass_kernel_spmd

    if "nc" not in _CACHE:
        _CACHE["nc"] = _build_nc()
    nc = _CACHE["nc"]

    x = np.asarray(x, dtype=np.float32)
    wq = np.asarray(wq, dtype=np.float32)
    wk = np.asarray(wk, dtype=np.float32)
    wv = np.asarray(wv, dtype=np.float32)
    wo = np.asarray(wo, dtype=np.float32)

    in_maps = []
    for c in range(8):
        b, hp = divmod(c, 4)
        e0 = hp * P
        in_maps.append({
            "xT": np.ascontiguousarray(x[b].T.astype(np.float16)),
            "wqS": _wsb(wq, e0),
            "wkS": _wsb(wk, e0),
            "wvS": _wsb(wv, e0),
            "woT0": np.ascontiguousarray(wo[:, e0:e0 + DK].T.astype(np.float16)),
            "woT1": np.ascontiguousarray(wo[:, e0 + DK:e0 + P].T.astype(np.float16)),
        })

    res = run_bass_kernel_spmd(
        nc, in_maps, core_ids=list(range(8)), trace=TRACE, tmpdir=TMPDIR
    )
    LAST_RESULTS = res

    y = np.zeros((B, S, D), dtype=np.float32)
    for c in range(8):
        y[c // 4] += res.results[c]["yT"].T
    return y



# revision 31
# speedup vs baseline: 1.0147x; 1.0009x over previous
# Multi-head self-attention (B=2, S=4096, D=512, H=8) on 8 NeuronCores.
#
# Sharding: core c -> batch b = c//4, head-pair hp = c%4 (heads 2hp, 2hp+1,
# i.e. channels [128*hp, 128*hp+128) of the QKV projection space).
# Host pre-slices/transposes weights + x per core (cast fp16 for the PE);
# device does all matmuls (QKV projections, flash-attention with fused
# softmax, W_O row-slice projection); host sums the 4 per-core W_O partials
# per batch (the "all-reduce") and transposes back.
#
# Per-core device kernel (matmul operands fp16, accumulation fp32 PSUM):
#   qtz_h/ktz_h [128, 4096]: Q^T/K^T per head, dk on a 64-partition band,
#     zero elsewhere -> every attention matmul is full-K (128,128) mode.
#   scoresT[kpos, q] = K Q^T chunkwise -> ACT exp(x/8) straight from PSUM
#   AV with a ones-column appended to V -> denominator for free
#   softmax division off the critical path (DVE recip + gpsimd broadcast).
# All pools stay open for the whole kernel; PSUM slots are shared between
# phases via tags (8 banks total) so phases overlap with per-slot WAR deps
# instead of pool-close barriers.

import numpy as np

B, S, D, H, DK = 2, 4096, 512, 8, 64
# Schraudolph fp16 exp: bits = round(A*s + B) read as fp16 ~= exp(s/8);
# 0.0430357 centers the multiplicative error of the linear-mantissa
# approximation at +-3%.
EXP_A = 1024.0 * 1.4426950408889634 / 8.0
EXP_B = 15360.0 - 1024.0 * 0.04303566
P = 128          # partition tile
NQ = 512         # matmul moving free dim (one fp32 PSUM bank)
QCH = 1024       # q-chunk (2 x NQ) => one [128,1024] exp per kpos-chunk
NKC = S // P     # kpos chunks (32)
NST = S // NQ    # s-tiles of 512 (8)
NDC = D // P     # d chunks (4)
NQC = S // QCH   # q chunks (4)

TRACE = False            # test.py sets True to get exec_time_ns + perfetto
TMPDIR = None            # optional trace output dir
LAST_RESULTS = None      # BassKernelResults of the last run (for test.py)

_CACHE = {}


def _build_nc():
    import concourse.bass as bass  # noqa: F401
    import concourse.mybir as mybir
    import concourse.tile as tile
    from concourse import bacc
    from concourse.masks import make_identity

    f32 = mybir.dt.float32
    f16 = mybir.dt.float16
    i16 = mybir.dt.int16
    Act = mybir.ActivationFunctionType
    Alu = mybir.AluOpType

    nc = bacc.Bacc("TRN2", target_bir_lowering=False, debug=False, num_devices=8)

    xT = nc.dram_tensor("xT", [D, S], f16, kind="ExternalInput")
    # weight slices pre-arranged by the host in the exact SBUF layout
    # ([P, D] row-slice of the torch weight) -> one contiguous DMA each
    wqS = nc.dram_tensor("wqS", [P, D], f16, kind="ExternalInput")
    wkS = nc.dram_tensor("wkS", [P, D], f16, kind="ExternalInput")
    wvS = nc.dram_tensor("wvS", [P, D], f16, kind="ExternalInput")
    woT0 = nc.dram_tensor("woT0", [DK, D], f16, kind="ExternalInput")
    woT1 = nc.dram_tensor("woT1", [DK, D], f16, kind="ExternalInput")
    yT = nc.dram_tensor("yT", [D, S], f32, kind="ExternalOutput")

    with tile.TileContext(nc) as tc:
        with (
            tc.tile_pool(name="sb", bufs=1) as sb,
            tc.tile_pool(name="ps", bufs=1, space="PSUM") as psp,
        ):
            # PSUM budget (8 banks total, slots shared across phases by tag):
            #   sc0, sc1: [128,1024] -> 2 banks each (scores / exp staging)
            #   av00..av11: [128,512] -> 1 bank each (AV accum; also used by
            #   the QKV-projection psum tiles and the W_O psum tiles)
            def av_ps(i, shape):
                return psp.tile(shape, f32, tag=f"av{i % 4}", name=f"avps{i}")

            # ---- persistent operand tiles -----------------------------------
            qtz = [sb.tile([P, S], f16, tag=f"qtz{h}", name=f"qtz{h}")
                   for h in range(2)]
            ktz = [sb.tile([P, S], f16, tag=f"ktz{h}", name=f"ktz{h}")
                   for h in range(2)]
            vb = [sb.tile([P, NKC * (DK + 1)], f16, tag=f"vb{h}", name=f"vb{h}")
                  for h in range(2)]
            outtz = [sb.tile([P, S], f16, tag=f"outtz{h}", name=f"outtz{h}")
                     for h in range(2)]
            wosz = [sb.tile([P, D], f16, tag=f"wosz{h}", name=f"wosz{h}")
                    for h in range(2)]

            # zero bands + ones-fill on the (otherwise idle) gpsimd engine;
            # bands first: the first scores matmul needs them, vb is needed
            # slightly later by the first AV matmul
            nc.gpsimd.memset(qtz[0][DK:P, :], 0.0)
            nc.gpsimd.memset(ktz[0][DK:P, :], 0.0)
            nc.gpsimd.memset(qtz[1][0:DK, :], 0.0)
            nc.gpsimd.memset(ktz[1][0:DK, :], 0.0)
            nc.gpsimd.memset(vb[0][:, :], 1.0)
            nc.gpsimd.memset(vb[1][:, :], 1.0)
            nc.gpsimd.memset(outtz[0][DK:P, :], 0.0)
            nc.gpsimd.memset(outtz[1][DK:P, :], 0.0)
            nc.gpsimd.memset(wosz[0][DK:P, :], 0.0)
            nc.gpsimd.memset(wosz[1][DK:P, :], 0.0)

            # ---- phase 1: load x + weights, QKV projections, build V -------
            xts = [sb.tile([P, S], f16, tag=f"xt{dc}", name=f"xt{dc}")
                   for dc in range(NDC)]
            wsb = {}
            for name, dram in (("v", wvS), ("k", wkS), ("q", wqS)):
                w = sb.tile([P, NDC * P], f16, tag=f"w{name}", name=f"w{name}")
                nc.sync.dma_start(w[:, :], dram[:, :])
                wsb[name] = w
            nc.sync.dma_start(wosz[0][0:DK, :], woT0[:, :])
            nc.sync.dma_start(wosz[1][0:DK, :], woT1[:, :])
            # x in two wide transfers per d-chunk: few triggers (the ~0.7us
            # per-descriptor sync cost dominated startup with 32 small ones),
            # while the half split still lets s-tiles 0-3 start early
            for quart in range(4):
                hs = slice(quart * (S // 4), (quart + 1) * (S // 4))
                for dc in range(NDC):
                    nc.sync.dma_start(xts[dc][:, hs], xT[dc * P:(dc + 1) * P, hs])

            vt = sb.tile([P, S], f32, tag="vt")
            ident = sb.tile([P, P], f32, tag="ident")
            make_identity(nc, ident[:, :])

            psn = 0
            for st in range(NST):
                for name in ("v", "k", "q"):
                    w = wsb[name]
                    ps = av_ps(psn, [P, NQ])
                    psn += 1
                    for dc in range(NDC):
                        nc.tensor.matmul(
                            ps[:, :],
                            w[:, dc * P:(dc + 1) * P],
                            xts[dc][:, st * NQ:(st + 1) * NQ],
                            start=(dc == 0),
                            stop=(dc == NDC - 1),
                        )
                    sl = slice(st * NQ, (st + 1) * NQ)
                    if name == "v":
                        nc.vector.tensor_copy(vt[:, sl], ps[:, :])
                        # transpose this V window into per-head V chunks
                        # [kpos, dk] (+ones col at 64) right away
                        for ch in range(4 * st, 4 * st + 4):
                            tp = av_ps(psn, [P, P])
                            psn += 1
                            nc.tensor.transpose(
                                tp[:, :], vt[:, ch * P:(ch + 1) * P], ident[:, :]
                            )
                            c0 = ch * (DK + 1)
                            nc.scalar.copy(vb[0][:, c0:c0 + DK], tp[:, 0:DK])
                            nc.vector.tensor_copy(
                                vb[1][:, c0:c0 + DK], tp[:, DK:P]
                            )
                    elif name == "k":
                        nc.scalar.copy(ktz[0][0:DK, sl], ps[0:DK, :])
                        nc.scalar.copy(ktz[1][DK:P, sl], ps[DK:P, :])
                    else:
                        nc.vector.tensor_copy(qtz[0][0:DK, sl], ps[0:DK, :])
                        nc.vector.tensor_copy(qtz[1][DK:P, sl], ps[DK:P, :])

            # ---- phase 2: flash attention -----------------------------------
            def emit_normalize(qc, av):
                # evacuate av psum fast (frees the bank), then the softmax
                # division off the critical path in SBUF
                for h in range(2):
                    for sub in range(2):
                        a = av[h, sub]
                        raw = sb.tile([DK + 1, NQ], f32, tag=f"raw{h}{sub}",
                                      name=f"raw{h}{sub}", bufs=3)
                        nc.vector.tensor_copy(raw[:, :], a[0:DK + 1, :])
                        dn0 = sb.tile([P, NQ], f32, tag="dn0", bufs=4)
                        nc.sync.dma_start(dn0[0:1, :], raw[DK:DK + 1, :])
                        rc = sb.tile([P, NQ], f32, tag="rc", bufs=4)
                        nc.vector.reciprocal_approx_fast(rc[0:1, :], dn0[0:1, :])
                        rcb = sb.tile([DK, NQ], f32, tag="rcb", bufs=4)
                        nc.gpsimd.partition_broadcast(
                            rcb[:, :], rc[0:1, :], channels=DK
                        )
                        q0 = qc * QCH + sub * NQ
                        nc.vector.tensor_mul(
                            outtz[h][0:DK, q0:q0 + NQ], raw[0:DK, :], rcb[:, :]
                        )

            # W_O for one q-chunk's two s-tiles -- emitted right after
            # that chunk's normalize so the projection overlaps the next
            # chunk's attention instead of tailing the kernel
            def emit_wo(qc):
                nonlocal psn
                for st in (2 * qc, 2 * qc + 1):
                    for ec in range(NDC):
                        yp = av_ps(psn, [P, NQ])
                        psn += 1
                        for h in range(2):
                            nc.tensor.matmul(
                                yp[:, :],
                                wosz[h][:, ec * P:(ec + 1) * P],
                                outtz[h][:, st * NQ:(st + 1) * NQ],
                                start=(h == 0),
                                stop=(h == 1),
                            )
                        ys = sb.tile([P, NQ], f32, tag="ys", bufs=4)
                        nc.scalar.copy(ys[:, :], yp[:, :])
                        nc.sync.dma_start(
                            yT[ec * P:(ec + 1) * P, st * NQ:(st + 1) * NQ],
                            ys[:, :],
                        )

            pending = None
            for qc in range(NQC):
                av = {}
                for h in range(2):
                    for sub in range(2):
                        av[h, sub] = av_ps(psn, [P, NQ])
                        psn += 1
                for k in range(NKC):
                    if k == 3 and pending is not None:
                        emit_normalize(*pending)
                        emit_wo(pending[0])
                        pending = None
                    # one single-bank psum tile per scores matmul, so each
                    # exp sub-op depends on exactly one matmul and each WAR
                    # on exactly one exp op (no 2-bank tile-level coupling)
                    scps = [[psp.tile([P, NQ], f32, tag=f"sc{h}{sub}",
                                      name=f"sc{h}{sub}")
                             for sub in range(2)] for h in range(2)]
                    for h in range(2):
                        for sub in range(2):
                            q0 = qc * QCH + sub * NQ
                            nc.tensor.matmul(
                                scps[h][sub][:, :],
                                ktz[h][:, k * P:(k + 1) * P],
                                qtz[h][:, q0:q0 + NQ],
                                start=True,
                                stop=True,
                            )
                    # exp: head0 on ACT (true exp), head1 on the DVE as a
                    # Schraudolph bit-trick exp -- int16(A*s+B) whose bit
                    # pattern read as fp16 is exp(s/8) within ~+-3%
                    # (mean-free; the softmax ratio cancels most of it)
                    ex0 = sb.tile([P, QCH], f16, tag="ex0", name="ex0", bufs=6)
                    ex1i = sb.tile([P, QCH], i16, tag="ex1", name="ex1", bufs=6)
                    for sub in range(2):
                        ssl = slice(sub * NQ, (sub + 1) * NQ)
                        nc.scalar.activation(
                            ex0[:, ssl], scps[0][sub][:, :], Act.Exp,
                            scale=0.125
                        )
                        nc.vector.tensor_scalar(
                            ex1i[:, ssl], scps[1][sub][:, :], EXP_A, EXP_B,
                            Alu.mult, Alu.add,
                        )
                    exs = [ex0, ex1i.bitcast(f16)]
                    c0 = k * (DK + 1)
                    for h in range(2):
                        for sub in range(2):
                            nc.tensor.matmul(
                                av[h, sub][0:DK + 1, :],
                                vb[h][:, c0:c0 + DK + 1],
                                exs[h][:, sub * NQ:(sub + 1) * NQ],
                                start=(k == 0),
                                stop=(k == NKC - 1),
                            )
                pending = (qc, av)
            emit_normalize(*pending)
            emit_wo(pending[0])

    nc.compile()
    return nc


def _wsb(w, e0):
    # SBUF weight layout: w_sb[p, dc*P + m] = w[e0 + m, dc*P + p]
    wt = w[e0:e0 + P].T.astype(np.float16)          # [D, P]
    return np.ascontiguousarray(
        np.hstack([wt[d * P:(d + 1) * P, :] for d in range(NDC)])
    )


def kernel(x, wq, wk, wv, wo):
    global LAST_RESULTS
    from concourse.bass_utils import run_bass_kernel_spmd

    if "nc" not in _CACHE:
        _CACHE["nc"] = _build_nc()
    nc = _CACHE["nc"]

    x = np.asarray(x, dtype=np.float32)
    wq = np.asarray(wq, dtype=np.float32)
    wk = np.asarray(wk, dtype=np.float32)
    wv = np.asarray(wv, dtype=np.float32)
    wo = np.asarray(wo, dtype=np.float32)

    in_maps = []
    for c in range(8):
        b, hp = divmod(c, 4)
        e0 = hp * P
        in_maps.append({
            "xT": np.ascontiguousarray(x[b].T.astype(np.float16)),
            "wqS": _wsb(wq, e0),
            "wkS": _wsb(wk, e0),
            "wvS": _wsb(wv, e0),
            "woT0": np.ascontiguousarray(wo[:, e0:e0 + DK].T.astype(np.float16)),
            "woT1": np.ascontiguousarray(wo[:, e0 + DK:e0 + P].T.astype(np.float16)),
        })

    res = run_bass_kernel_spmd(
        nc, in_maps, core_ids=list(range(8)), trace=TRACE, tmpdir=TMPDIR
    )
    LAST_RESULTS = res

    y = np.zeros((B, S, D), dtype=np.float32)
    for c in range(8):
        y[c // 4] += res.results[c]["yT"].T
    return y



# revision 32
# speedup vs baseline: 1.0270x; 1.0121x over previous
# Multi-head self-attention (B=2, S=4096, D=512, H=8) on 8 NeuronCores.
#
# Sharding: core c -> batch b = c//4, head-pair hp = c%4 (heads 2hp, 2hp+1,
# i.e. channels [128*hp, 128*hp+128) of the QKV projection space).
# Host pre-slices/transposes weights + x per core (cast fp16 for the PE);
# device does all matmuls (QKV projections, flash-attention with fused
# softmax, W_O row-slice projection); host sums the 4 per-core W_O partials
# per batch (the "all-reduce") and transposes back.
#
# Per-core device kernel (matmul operands fp16, accumulation fp32 PSUM):
#   qtz_h/ktz_h [128, 4096]: Q^T/K^T per head, dk on a 64-partition band,
#     zero elsewhere -> every attention matmul is full-K (128,128) mode.
#   scoresT[kpos, q] = K Q^T chunkwise -> ACT exp(x/8) straight from PSUM
#   AV with a ones-column appended to V -> denominator for free
#   softmax division off the critical path (DVE recip + gpsimd broadcast).
# All pools stay open for the whole kernel; PSUM slots are shared between
# phases via tags (8 banks total) so phases overlap with per-slot WAR deps
# instead of pool-close barriers.

import numpy as np

B, S, D, H, DK = 2, 4096, 512, 8, 64
# Schraudolph fp16 exp: bits = round(A*s + B) read as fp16 ~= exp(s/8);
# 0.0430357 centers the multiplicative error of the linear-mantissa
# approximation at +-3%.
EXP_A = 1024.0 * 1.4426950408889634 / 8.0
EXP_B = 15360.0 - 1024.0 * 0.04303566
P = 128          # partition tile
NQ = 512         # matmul moving free dim (one fp32 PSUM bank)
QCH = 1024       # q-chunk (2 x NQ) => one [128,1024] exp per kpos-chunk
NKC = S // P     # kpos chunks (32)
NST = S // NQ    # s-tiles of 512 (8)
NDC = D // P     # d chunks (4)
NQC = S // QCH   # q chunks (4)

TRACE = False            # test.py sets True to get exec_time_ns + perfetto
TMPDIR = None            # optional trace output dir
LAST_RESULTS = None      # BassKernelResults of the last run (for test.py)

_CACHE = {}


def _build_nc():
    import concourse.bass as bass  # noqa: F401
    import concourse.mybir as mybir
    import concourse.tile as tile
    from concourse import bacc
    from concourse.masks import make_identity

    f32 = mybir.dt.float32
    f16 = mybir.dt.float16
    i16 = mybir.dt.int16
    Act = mybir.ActivationFunctionType
    Alu = mybir.AluOpType

    nc = bacc.Bacc("TRN2", target_bir_lowering=False, debug=False, num_devices=8)

    xT = nc.dram_tensor("xT", [D, S], f16, kind="ExternalInput")
    # weight slices pre-arranged by the host in the exact SBUF layout
    # ([P, D] row-slice of the torch weight) -> one contiguous DMA each
    wqS = nc.dram_tensor("wqS", [P, D], f16, kind="ExternalInput")
    wkS = nc.dram_tensor("wkS", [P, D], f16, kind="ExternalInput")
    wvS = nc.dram_tensor("wvS", [P, D], f16, kind="ExternalInput")
    woT0 = nc.dram_tensor("woT0", [DK, D], f16, kind="ExternalInput")
    woT1 = nc.dram_tensor("woT1", [DK, D], f16, kind="ExternalInput")
    yT = nc.dram_tensor("yT", [D, S], f32, kind="ExternalOutput")

    with tile.TileContext(nc) as tc:
        with (
            tc.tile_pool(name="sb", bufs=1) as sb,
            tc.tile_pool(name="ps", bufs=1, space="PSUM") as psp,
        ):
            # PSUM budget (8 banks total, slots shared across phases by tag):
            #   sc0, sc1: [128,1024] -> 2 banks each (scores / exp staging)
            #   av00..av11: [128,512] -> 1 bank each (AV accum; also used by
            #   the QKV-projection psum tiles and the W_O psum tiles)
            def av_ps(i, shape):
                return psp.tile(shape, f32, tag=f"av{i % 4}", name=f"avps{i}")

            # ---- persistent operand tiles -----------------------------------
            qtz = [sb.tile([P, S], f16, tag=f"qtz{h}", name=f"qtz{h}")
                   for h in range(2)]
            ktz = [sb.tile([P, S], f16, tag=f"ktz{h}", name=f"ktz{h}")
                   for h in range(2)]
            vb = [sb.tile([P, NKC * (DK + 1)], f16, tag=f"vb{h}", name=f"vb{h}")
                  for h in range(2)]
            outtz = [sb.tile([P, S], f16, tag=f"outtz{h}", name=f"outtz{h}")
                     for h in range(2)]
            wosz = [sb.tile([P, D], f16, tag=f"wosz{h}", name=f"wosz{h}")
                    for h in range(2)]

            # zero bands + ones-fill on the (otherwise idle) gpsimd engine;
            # bands first: the first scores matmul needs them, vb is needed
            # slightly later by the first AV matmul
            nc.gpsimd.memset(qtz[0][DK:P, :], 0.0)
            nc.gpsimd.memset(ktz[0][DK:P, :], 0.0)
            nc.gpsimd.memset(qtz[1][0:DK, :], 0.0)
            nc.gpsimd.memset(ktz[1][0:DK, :], 0.0)
            nc.gpsimd.memset(vb[0][:, :], 1.0)
            nc.gpsimd.memset(vb[1][:, :], 1.0)
            nc.gpsimd.memset(outtz[0][DK:P, :], 0.0)
            nc.gpsimd.memset(outtz[1][DK:P, :], 0.0)
            nc.gpsimd.memset(wosz[0][DK:P, :], 0.0)
            nc.gpsimd.memset(wosz[1][DK:P, :], 0.0)

            # ---- phase 1: load x + weights, QKV projections, build V -------
            xts = [sb.tile([P, S], f16, tag=f"xt{dc}", name=f"xt{dc}")
                   for dc in range(NDC)]
            # x first -- it is the startup critical path; quarter-tile
            # descriptors spread across DMA rings with fine-grained deps
            wsb = {}
            for name in ("v", "k", "q"):
                wsb[name] = sb.tile([P, NDC * P], f16, tag=f"w{name}",
                                    name=f"w{name}")
            for quart in range(4):
                hs = slice(quart * (S // 4), (quart + 1) * (S // 4))
                for dc in range(NDC):
                    nc.sync.dma_start(xts[dc][:, hs], xT[dc * P:(dc + 1) * P, hs])
            for name, dram in (("v", wvS), ("k", wkS), ("q", wqS)):
                nc.sync.dma_start(wsb[name][:, :], dram[:, :])
            nc.sync.dma_start(wosz[0][0:DK, :], woT0[:, :])
            nc.sync.dma_start(wosz[1][0:DK, :], woT1[:, :])

            vt = sb.tile([P, S], f32, tag="vt")
            ident = sb.tile([P, P], f32, tag="ident")
            make_identity(nc, ident[:, :])

            psn = 0
            for st in range(NST):
                for name in ("v", "k", "q"):
                    w = wsb[name]
                    ps = av_ps(psn, [P, NQ])
                    psn += 1
                    for dc in range(NDC):
                        nc.tensor.matmul(
                            ps[:, :],
                            w[:, dc * P:(dc + 1) * P],
                            xts[dc][:, st * NQ:(st + 1) * NQ],
                            start=(dc == 0),
                            stop=(dc == NDC - 1),
                        )
                    sl = slice(st * NQ, (st + 1) * NQ)
                    if name == "v":
                        nc.vector.tensor_copy(vt[:, sl], ps[:, :])
                        # transpose this V window into per-head V chunks
                        # [kpos, dk] (+ones col at 64) right away
                        for ch in range(4 * st, 4 * st + 4):
                            tp = av_ps(psn, [P, P])
                            psn += 1
                            nc.tensor.transpose(
                                tp[:, :], vt[:, ch * P:(ch + 1) * P], ident[:, :]
                            )
                            c0 = ch * (DK + 1)
                            nc.scalar.copy(vb[0][:, c0:c0 + DK], tp[:, 0:DK])
                            nc.vector.tensor_copy(
                                vb[1][:, c0:c0 + DK], tp[:, DK:P]
                            )
                    elif name == "k":
                        nc.scalar.copy(ktz[0][0:DK, sl], ps[0:DK, :])
                        nc.scalar.copy(ktz[1][DK:P, sl], ps[DK:P, :])
                    else:
                        nc.vector.tensor_copy(qtz[0][0:DK, sl], ps[0:DK, :])
                        nc.vector.tensor_copy(qtz[1][DK:P, sl], ps[DK:P, :])

            # ---- phase 2: flash attention -----------------------------------
            def emit_normalize(qc, av):
                # evacuate av psum fast (frees the bank), then the softmax
                # division off the critical path in SBUF; sub-major so each
                # s-tile's W_O can launch after ITS sub's muls
                for sub in range(2):
                    for h in range(2):
                        a = av[h, sub]
                        raw = sb.tile([DK + 1, NQ], f32, tag=f"raw{h}{sub}",
                                      name=f"raw{h}{sub}", bufs=3)
                        nc.vector.tensor_copy(raw[:, :], a[0:DK + 1, :])
                        dn0 = sb.tile([P, NQ], f32, tag="dn0", bufs=4)
                        nc.sync.dma_start(dn0[0:1, :], raw[DK:DK + 1, :])
                        rc = sb.tile([P, NQ], f32, tag="rc", bufs=4)
                        nc.vector.reciprocal_approx_fast(rc[0:1, :], dn0[0:1, :])
                        rcb = sb.tile([DK, NQ], f32, tag="rcb", bufs=4)
                        nc.gpsimd.partition_broadcast(
                            rcb[:, :], rc[0:1, :], channels=DK
                        )
                        q0 = qc * QCH + sub * NQ
                        nc.vector.tensor_mul(
                            outtz[h][0:DK, q0:q0 + NQ], raw[0:DK, :], rcb[:, :]
                        )
                    emit_wo_st(2 * qc + sub)

            # W_O for one q-chunk's two s-tiles -- emitted right after
            # that chunk's normalize so the projection overlaps the next
            # chunk's attention instead of tailing the kernel
            def emit_wo_st(st):
                nonlocal psn
                if True:
                    for ec in range(NDC):
                        yp = av_ps(psn, [P, NQ])
                        psn += 1
                        for h in range(2):
                            nc.tensor.matmul(
                                yp[:, :],
                                wosz[h][:, ec * P:(ec + 1) * P],
                                outtz[h][:, st * NQ:(st + 1) * NQ],
                                start=(h == 0),
                                stop=(h == 1),
                            )
                        ys = sb.tile([P, NQ], f32, tag="ys", bufs=4)
                        nc.scalar.copy(ys[:, :], yp[:, :])
                        nc.sync.dma_start(
                            yT[ec * P:(ec + 1) * P, st * NQ:(st + 1) * NQ],
                            ys[:, :],
                        )

            pending = None
            for qc in range(NQC):
                av = {}
                for h in range(2):
                    for sub in range(2):
                        av[h, sub] = av_ps(psn, [P, NQ])
                        psn += 1
                for k in range(NKC):
                    if k == 3 and pending is not None:
                        emit_normalize(*pending)
                        pending = None
                    # one single-bank psum tile per scores matmul, so each
                    # exp sub-op depends on exactly one matmul and each WAR
                    # on exactly one exp op (no 2-bank tile-level coupling)
                    scps = [[psp.tile([P, NQ], f32, tag=f"sc{h}{sub}",
                                      name=f"sc{h}{sub}")
                             for sub in range(2)] for h in range(2)]
                    for h in range(2):
                        for sub in range(2):
                            q0 = qc * QCH + sub * NQ
                            nc.tensor.matmul(
                                scps[h][sub][:, :],
                                ktz[h][:, k * P:(k + 1) * P],
                                qtz[h][:, q0:q0 + NQ],
                                start=True,
                                stop=True,
                            )
                    # exp: head0 on ACT (true exp), head1 on the DVE as a
                    # Schraudolph bit-trick exp -- int16(A*s+B) whose bit
                    # pattern read as fp16 is exp(s/8) within ~+-3%
                    # (mean-free; the softmax ratio cancels most of it)
                    ex0 = sb.tile([P, QCH], f16, tag="ex0", name="ex0", bufs=6)
                    ex1i = sb.tile([P, QCH], i16, tag="ex1", name="ex1", bufs=6)
                    for sub in range(2):
                        ssl = slice(sub * NQ, (sub + 1) * NQ)
                        nc.scalar.activation(
                            ex0[:, ssl], scps[0][sub][:, :], Act.Exp,
                            scale=0.125
                        )
                        nc.vector.tensor_scalar(
                            ex1i[:, ssl], scps[1][sub][:, :], EXP_A, EXP_B,
                            Alu.mult, Alu.add,
                        )
                    exs = [ex0, ex1i.bitcast(f16)]
                    c0 = k * (DK + 1)
                    for h in range(2):
                        for sub in range(2):
                            nc.tensor.matmul(
                                av[h, sub][0:DK + 1, :],
                                vb[h][:, c0:c0 + DK + 1],
                                exs[h][:, sub * NQ:(sub + 1) * NQ],
                                start=(k == 0),
                                stop=(k == NKC - 1),
                            )
                pending = (qc, av)
            emit_normalize(*pending)

    nc.compile()
    return nc


def _wsb(w, e0):
    # SBUF weight layout: w_sb[p, dc*P + m] = w[e0 + m, dc*P + p]
    wt = w[e0:e0 + P].T.astype(np.float16)          # [D, P]
    return np.ascontiguousarray(
        np.hstack([wt[d * P:(d + 1) * P, :] for d in range(NDC)])
    )


def kernel(x, wq, wk, wv, wo):
    global LAST_RESULTS
    from concourse.bass_utils import run_bass_kernel_spmd

    if "nc" not in _CACHE:
        _CACHE["nc"] = _build_nc()
    nc = _CACHE["nc"]

    x = np.asarray(x, dtype=np.float32)
    wq = np.asarray(wq, dtype=np.float32)
    wk = np.asarray(wk, dtype=np.float32)
    wv = np.asarray(wv, dtype=np.float32)
    wo = np.asarray(wo, dtype=np.float32)

    in_maps = []
    for c in range(8):
        b, hp = divmod(c, 4)
        e0 = hp * P
        in_maps.append({
            "xT": np.ascontiguousarray(x[b].T.astype(np.float16)),
            "wqS": _wsb(wq, e0),
            "wkS": _wsb(wk, e0),
            "wvS": _wsb(wv, e0),
            "woT0": np.ascontiguousarray(wo[:, e0:e0 + DK].T.astype(np.float16)),
            "woT1": np.ascontiguousarray(wo[:, e0 + DK:e0 + P].T.astype(np.float16)),
        })

    res = run_bass_kernel_spmd(
        nc, in_maps, core_ids=list(range(8)), trace=TRACE, tmpdir=TMPDIR
    )
    LAST_RESULTS = res

    y = np.zeros((B, S, D), dtype=np.float32)
    for c in range(8):
        y[c // 4] += res.results[c]["yT"].T
    return y



# revision 33
# speedup vs baseline: 1.0467x; 1.0192x over previous
# Multi-head self-attention (B=2, S=4096, D=512, H=8) on 8 NeuronCores.
#
# Sharding: core c -> batch b = c//4, head-pair hp = c%4 (heads 2hp, 2hp+1,
# i.e. channels [128*hp, 128*hp+128) of the QKV projection space).
# Host pre-slices/transposes weights + x per core (cast fp16 for the PE);
# device does all matmuls (QKV projections, flash-attention with fused
# softmax, W_O row-slice projection); host sums the 4 per-core W_O partials
# per batch (the "all-reduce") and transposes back.
#
# Per-core device kernel (matmul operands fp16, accumulation fp32 PSUM):
#   qtz_h/ktz_h [128, 4096]: Q^T/K^T per head, dk on a 64-partition band,
#     zero elsewhere -> every attention matmul is full-K (128,128) mode.
#   scoresT[kpos, q] = K Q^T chunkwise -> ACT exp(x/8) straight from PSUM
#   AV with a ones-column appended to V -> denominator for free
#   softmax division off the critical path (DVE recip + gpsimd broadcast).
# All pools stay open for the whole kernel; PSUM slots are shared between
# phases via tags (8 banks total) so phases overlap with per-slot WAR deps
# instead of pool-close barriers.

import numpy as np

B, S, D, H, DK = 2, 4096, 512, 8, 64
# Schraudolph fp16 exp: bits = round(A*s + B) read as fp16 ~= exp(s/8);
# 0.0430357 centers the multiplicative error of the linear-mantissa
# approximation at +-3%.
EXP_A = 1024.0 * 1.4426950408889634 / 8.0
EXP_B = 15360.0 - 1024.0 * 0.04303566
P = 128          # partition tile
NQ = 512         # matmul moving free dim (one fp32 PSUM bank)
QCH = 1024       # q-chunk (2 x NQ) => one [128,1024] exp per kpos-chunk
NKC = S // P     # kpos chunks (32)
NST = S // NQ    # s-tiles of 512 (8)
NDC = D // P     # d chunks (4)
NQC = S // QCH   # q chunks (4)

TRACE = False            # test.py sets True to get exec_time_ns + perfetto
TMPDIR = None            # optional trace output dir
LAST_RESULTS = None      # BassKernelResults of the last run (for test.py)

_CACHE = {}


def _build_nc():
    import concourse.bass as bass  # noqa: F401
    import concourse.mybir as mybir
    import concourse.tile as tile
    from concourse import bacc
    from concourse.masks import make_identity

    f32 = mybir.dt.float32
    f16 = mybir.dt.float16
    i16 = mybir.dt.int16
    Act = mybir.ActivationFunctionType
    Alu = mybir.AluOpType

    nc = bacc.Bacc("TRN2", target_bir_lowering=False, debug=False, num_devices=8)

    xT = nc.dram_tensor("xT", [D, S], f16, kind="ExternalInput")
    # weight slices pre-arranged by the host in the exact SBUF layout
    # ([P, D] row-slice of the torch weight) -> one contiguous DMA each
    wqS = nc.dram_tensor("wqS", [P, D], f16, kind="ExternalInput")
    wkS = nc.dram_tensor("wkS", [P, D], f16, kind="ExternalInput")
    wvS = nc.dram_tensor("wvS", [P, D], f16, kind="ExternalInput")
    woT0 = nc.dram_tensor("woT0", [DK, D], f16, kind="ExternalInput")
    woT1 = nc.dram_tensor("woT1", [DK, D], f16, kind="ExternalInput")
    yT = nc.dram_tensor("yT", [D, S], f32, kind="ExternalOutput")

    with tile.TileContext(nc) as tc:
        with (
            tc.tile_pool(name="sb", bufs=1) as sb,
            tc.tile_pool(name="ps", bufs=1, space="PSUM") as psp,
        ):
            # PSUM budget (8 banks total, slots shared across phases by tag):
            #   sc0, sc1: [128,1024] -> 2 banks each (scores / exp staging)
            #   av00..av11: [128,512] -> 1 bank each (AV accum; also used by
            #   the QKV-projection psum tiles and the W_O psum tiles)
            def av_ps(i, shape):
                return psp.tile(shape, f32, tag=f"av{i % 4}", name=f"avps{i}")

            # ---- persistent operand tiles -----------------------------------
            qtz = [sb.tile([P, S], f16, tag=f"qtz{h}", name=f"qtz{h}")
                   for h in range(2)]
            ktz = [sb.tile([P, S], f16, tag=f"ktz{h}", name=f"ktz{h}")
                   for h in range(2)]
            vb = [sb.tile([P, NKC * (DK + 1)], f16, tag=f"vb{h}", name=f"vb{h}")
                  for h in range(2)]
            outtz = [sb.tile([P, S], f16, tag=f"outtz{h}", name=f"outtz{h}")
                     for h in range(2)]
            wosz = [sb.tile([P, D], f16, tag=f"wosz{h}", name=f"wosz{h}")
                    for h in range(2)]

            # zero bands + ones-fill on the (otherwise idle) gpsimd engine;
            # bands first: the first scores matmul needs them, vb is needed
            # slightly later by the first AV matmul
            nc.gpsimd.memset(qtz[0][DK:P, :], 0.0)
            nc.gpsimd.memset(ktz[0][DK:P, :], 0.0)
            nc.gpsimd.memset(qtz[1][0:DK, :], 0.0)
            nc.gpsimd.memset(ktz[1][0:DK, :], 0.0)
            nc.gpsimd.memset(vb[0][:, :], 1.0)
            nc.gpsimd.memset(vb[1][:, :], 1.0)
            nc.gpsimd.memset(outtz[0][DK:P, :], 0.0)
            nc.gpsimd.memset(outtz[1][DK:P, :], 0.0)
            nc.gpsimd.memset(wosz[0][DK:P, :], 0.0)
            nc.gpsimd.memset(wosz[1][DK:P, :], 0.0)

            # ---- phase 1: load x + weights, QKV projections, build V -------
            xts = [sb.tile([P, S], f16, tag=f"xt{dc}", name=f"xt{dc}")
                   for dc in range(NDC)]
            # x first -- it is the startup critical path; quarter-tile
            # descriptors spread across DMA rings with fine-grained deps
            wsb = {}
            for name in ("v", "k", "q"):
                wsb[name] = sb.tile([P, NDC * P], f16, tag=f"w{name}",
                                    name=f"w{name}")
            for quart in range(4):
                hs = slice(quart * (S // 4), (quart + 1) * (S // 4))
                for dc in range(NDC):
                    nc.sync.dma_start(xts[dc][:, hs], xT[dc * P:(dc + 1) * P, hs])
            for name, dram in (("v", wvS), ("k", wkS), ("q", wqS)):
                nc.sync.dma_start(wsb[name][:, :], dram[:, :])
            nc.sync.dma_start(wosz[0][0:DK, :], woT0[:, :])
            nc.sync.dma_start(wosz[1][0:DK, :], woT1[:, :])

            vt = sb.tile([P, S], f32, tag="vt")
            ident = sb.tile([P, P], f32, tag="ident")
            make_identity(nc, ident[:, :])

            psn = 0
            for st in range(NST):
                for name in ("v", "k", "q"):
                    w = wsb[name]
                    ps = av_ps(psn, [P, NQ])
                    psn += 1
                    for dc in range(NDC):
                        nc.tensor.matmul(
                            ps[:, :],
                            w[:, dc * P:(dc + 1) * P],
                            xts[dc][:, st * NQ:(st + 1) * NQ],
                            start=(dc == 0),
                            stop=(dc == NDC - 1),
                        )
                    sl = slice(st * NQ, (st + 1) * NQ)
                    if name == "v":
                        nc.vector.tensor_copy(vt[:, sl], ps[:, :])
                        # transpose this V window into per-head V chunks
                        # [kpos, dk] (+ones col at 64) right away
                        for ch in range(4 * st, 4 * st + 4):
                            tp = av_ps(psn, [P, P])
                            psn += 1
                            nc.tensor.transpose(
                                tp[:, :], vt[:, ch * P:(ch + 1) * P], ident[:, :]
                            )
                            c0 = ch * (DK + 1)
                            nc.scalar.copy(vb[0][:, c0:c0 + DK], tp[:, 0:DK])
                            nc.vector.tensor_copy(
                                vb[1][:, c0:c0 + DK], tp[:, DK:P]
                            )
                    elif name == "k":
                        nc.scalar.copy(ktz[0][0:DK, sl], ps[0:DK, :])
                        nc.scalar.copy(ktz[1][DK:P, sl], ps[DK:P, :])
                    else:
                        nc.vector.tensor_copy(qtz[0][0:DK, sl], ps[0:DK, :])
                        nc.vector.tensor_copy(qtz[1][DK:P, sl], ps[DK:P, :])

            # ---- phase 2: flash attention -----------------------------------
            def emit_normalize(qc, av):
                # evacuate av psum fast (frees the bank), then the softmax
                # division off the critical path in SBUF; sub-major so each
                # s-tile's W_O can launch after ITS sub's muls
                for sub in range(2):
                    for h in range(2):
                        a = av[h, sub]
                        raw = sb.tile([DK + 1, NQ], f32, tag=f"raw{h}{sub}",
                                      name=f"raw{h}{sub}", bufs=3)
                        nc.vector.tensor_copy(raw[:, :], a[0:DK + 1, :])
                        dn0 = sb.tile([P, NQ], f32, tag="dn0", bufs=4)
                        nc.sync.dma_start(dn0[0:1, :], raw[DK:DK + 1, :])
                        rc = sb.tile([P, NQ], f32, tag="rc", bufs=4)
                        nc.vector.reciprocal_approx_fast(rc[0:1, :], dn0[0:1, :])
                        rcb = sb.tile([DK, NQ], f32, tag="rcb", bufs=4)
                        nc.gpsimd.partition_broadcast(
                            rcb[:, :], rc[0:1, :], channels=DK
                        )
                        q0 = qc * QCH + sub * NQ
                        nc.vector.tensor_mul(
                            outtz[h][0:DK, q0:q0 + NQ], raw[0:DK, :], rcb[:, :]
                        )
                    emit_wo_st(2 * qc + sub)

            # W_O for one q-chunk's two s-tiles -- emitted right after
            # that chunk's normalize so the projection overlaps the next
            # chunk's attention instead of tailing the kernel
            def emit_wo_st(st):
                nonlocal psn
                if True:
                    for ec in range(NDC):
                        yp = av_ps(psn, [P, NQ])
                        psn += 1
                        for h in range(2):
                            nc.tensor.matmul(
                                yp[:, :],
                                wosz[h][:, ec * P:(ec + 1) * P],
                                outtz[h][:, st * NQ:(st + 1) * NQ],
                                start=(h == 0),
                                stop=(h == 1),
                            )
                        ys = sb.tile([P, NQ], f32, tag="ys", bufs=4)
                        nc.scalar.copy(ys[:, :], yp[:, :])
                        nc.sync.dma_start(
                            yT[ec * P:(ec + 1) * P, st * NQ:(st + 1) * NQ],
                            ys[:, :],
                        )

            pending = None
            av_pending = None

            def emit_av(k, exs):
                c0 = k * (DK + 1)
                for h in range(2):
                    for sub in range(2):
                        nc.tensor.matmul(
                            av[h, sub][0:DK + 1, :],
                            vb[h][:, c0:c0 + DK + 1],
                            exs[h][:, sub * NQ:(sub + 1) * NQ],
                            start=(k == 0),
                            stop=(k == NKC - 1),
                        )

            for qc in range(NQC):
                av = {}
                for h in range(2):
                    for sub in range(2):
                        av[h, sub] = av_ps(psn, [P, NQ])
                        psn += 1
                for k in range(NKC):
                    if k == 3 and pending is not None:
                        emit_normalize(*pending)
                        pending = None
                    # one single-bank psum tile per scores matmul, so each
                    # exp sub-op depends on exactly one matmul and each WAR
                    # on exactly one exp op (no 2-bank tile-level coupling)
                    scps = [[psp.tile([P, NQ], f32, tag=f"sc{h}{sub}",
                                      name=f"sc{h}{sub}")
                             for sub in range(2)] for h in range(2)]
                    for h in range(2):
                        for sub in range(2):
                            q0 = qc * QCH + sub * NQ
                            nc.tensor.matmul(
                                scps[h][sub][:, :],
                                ktz[h][:, k * P:(k + 1) * P],
                                qtz[h][:, q0:q0 + NQ],
                                start=True,
                                stop=True,
                            )
                    # exp: head0 on ACT (true exp), head1 on the DVE as a
                    # Schraudolph bit-trick exp -- int16(A*s+B) whose bit
                    # pattern read as fp16 is exp(s/8) within ~+-3%
                    # (mean-free; the softmax ratio cancels most of it)
                    ex0 = sb.tile([P, QCH], f16, tag="ex0", name="ex0", bufs=6)
                    ex1i = sb.tile([P, QCH], i16, tag="ex1", name="ex1", bufs=6)
                    for sub in range(2):
                        ssl = slice(sub * NQ, (sub + 1) * NQ)
                        nc.scalar.activation(
                            ex0[:, ssl], scps[0][sub][:, :], Act.Exp,
                            scale=0.125
                        )
                        nc.vector.tensor_scalar(
                            ex1i[:, ssl], scps[1][sub][:, :], EXP_A, EXP_B,
                            Alu.mult, Alu.add,
                        )
                    exs = [ex0, ex1i.bitcast(f16)]
                    # defer this chunk's AV by one chunk: when emitted, all
                    # its exp inputs are long done, so the PE never waits on
                    # the chunk's last exp op (pins the period to the matmul
                    # streaming floor)
                    if av_pending is not None:
                        emit_av(*av_pending)
                    av_pending = (k, exs)
                for k2, exs2 in (av_pending,):
                    emit_av(k2, exs2)
                av_pending = None
                pending = (qc, av)
            emit_normalize(*pending)

    nc.compile()
    return nc


def _wsb(w, e0):
    # SBUF weight layout: w_sb[p, dc*P + m] = w[e0 + m, dc*P + p]
    wt = w[e0:e0 + P].T.astype(np.float16)          # [D, P]
    return np.ascontiguousarray(
        np.hstack([wt[d * P:(d + 1) * P, :] for d in range(NDC)])
    )


def kernel(x, wq, wk, wv, wo):
    global LAST_RESULTS
    from concourse.bass_utils import run_bass_kernel_spmd

    if "nc" not in _CACHE:
        _CACHE["nc"] = _build_nc()
    nc = _CACHE["nc"]

    x = np.asarray(x, dtype=np.float32)
    wq = np.asarray(wq, dtype=np.float32)
    wk = np.asarray(wk, dtype=np.float32)
    wv = np.asarray(wv, dtype=np.float32)
    wo = np.asarray(wo, dtype=np.float32)

    in_maps = []
    for c in range(8):
        b, hp = divmod(c, 4)
        e0 = hp * P
        in_maps.append({
            "xT": np.ascontiguousarray(x[b].T.astype(np.float16)),
            "wqS": _wsb(wq, e0),
            "wkS": _wsb(wk, e0),
            "wvS": _wsb(wv, e0),
            "woT0": np.ascontiguousarray(wo[:, e0:e0 + DK].T.astype(np.float16)),
            "woT1": np.ascontiguousarray(wo[:, e0 + DK:e0 + P].T.astype(np.float16)),
        })

    res = run_bass_kernel_spmd(
        nc, in_maps, core_ids=list(range(8)), trace=TRACE, tmpdir=TMPDIR
    )
    LAST_RESULTS = res

    y = np.zeros((B, S, D), dtype=np.float32)
    for c in range(8):
        y[c // 4] += res.results[c]["yT"].T
    return y



# revision 34
# speedup vs baseline: 1.0494x; 1.0025x over previous
# Multi-head self-attention (B=2, S=4096, D=512, H=8) on 8 NeuronCores.
#
# Sharding: core c -> batch b = c//4, head-pair hp = c%4 (heads 2hp, 2hp+1,
# i.e. channels [128*hp, 128*hp+128) of the QKV projection space).
# Host pre-slices/transposes weights + x per core (cast fp16 for the PE);
# device does all matmuls (QKV projections, flash-attention with fused
# softmax, W_O row-slice projection); host sums the 4 per-core W_O partials
# per batch (the "all-reduce") and transposes back.
#
# Per-core device kernel (matmul operands fp16, accumulation fp32 PSUM):
#   qtz_h/ktz_h [128, 4096]: Q^T/K^T per head, dk on a 64-partition band,
#     zero elsewhere -> every attention matmul is full-K (128,128) mode.
#   scoresT[kpos, q] = K Q^T chunkwise -> ACT exp(x/8) straight from PSUM
#   AV with a ones-column appended to V -> denominator for free
#   softmax division off the critical path (DVE recip + gpsimd broadcast).
# All pools stay open for the whole kernel; PSUM slots are shared between
# phases via tags (8 banks total) so phases overlap with per-slot WAR deps
# instead of pool-close barriers.

import numpy as np

B, S, D, H, DK = 2, 4096, 512, 8, 64
# Schraudolph fp16 exp: bits = round(A*s + B) read as fp16 ~= exp(s/8);
# 0.0430357 centers the multiplicative error of the linear-mantissa
# approximation at +-3%.
EXP_A = 1024.0 * 1.4426950408889634 / 8.0
EXP_B = 15360.0 - 1024.0 * 0.04303566
P = 128          # partition tile
NQ = 512         # matmul moving free dim (one fp32 PSUM bank)
QCH = 1024       # q-chunk (2 x NQ) => one [128,1024] exp per kpos-chunk
NKC = S // P     # kpos chunks (32)
NST = S // NQ    # s-tiles of 512 (8)
NDC = D // P     # d chunks (4)
NQC = S // QCH   # q chunks (4)

TRACE = False            # test.py sets True to get exec_time_ns + perfetto
TMPDIR = None            # optional trace output dir
LAST_RESULTS = None      # BassKernelResults of the last run (for test.py)

_CACHE = {}


def _build_nc():
    import concourse.bass as bass  # noqa: F401
    import concourse.mybir as mybir
    import concourse.tile as tile
    from concourse import bacc
    from concourse.masks import make_identity

    f32 = mybir.dt.float32
    f16 = mybir.dt.float16
    i16 = mybir.dt.int16
    Act = mybir.ActivationFunctionType
    Alu = mybir.AluOpType

    nc = bacc.Bacc("TRN2", target_bir_lowering=False, debug=False, num_devices=8)

    xT = nc.dram_tensor("xT", [D, S], f16, kind="ExternalInput")
    # weight slices pre-arranged by the host in the exact SBUF layout
    # ([P, D] row-slice of the torch weight) -> one contiguous DMA each
    wqS = nc.dram_tensor("wqS", [P, D], f16, kind="ExternalInput")
    wkS = nc.dram_tensor("wkS", [P, D], f16, kind="ExternalInput")
    wvS = nc.dram_tensor("wvS", [P, D], f16, kind="ExternalInput")
    woT0 = nc.dram_tensor("woT0", [DK, D], f16, kind="ExternalInput")
    woT1 = nc.dram_tensor("woT1", [DK, D], f16, kind="ExternalInput")
    yT = nc.dram_tensor("yT", [D, S], f32, kind="ExternalOutput")

    with tile.TileContext(nc) as tc:
        with (
            tc.tile_pool(name="sb", bufs=1) as sb,
            tc.tile_pool(name="ps", bufs=1, space="PSUM") as psp,
        ):
            # PSUM budget (8 banks total, slots shared across phases by tag):
            #   sc0, sc1: [128,1024] -> 2 banks each (scores / exp staging)
            #   av00..av11: [128,512] -> 1 bank each (AV accum; also used by
            #   the QKV-projection psum tiles and the W_O psum tiles)
            def av_ps(i, shape):
                return psp.tile(shape, f32, tag=f"av{i % 4}", name=f"avps{i}")

            # ---- persistent operand tiles -----------------------------------
            qtz = [sb.tile([P, S], f16, tag=f"qtz{h}", name=f"qtz{h}")
                   for h in range(2)]
            ktz = [sb.tile([P, S], f16, tag=f"ktz{h}", name=f"ktz{h}")
                   for h in range(2)]
            vb = [sb.tile([P, NKC * (DK + 1)], f16, tag=f"vb{h}", name=f"vb{h}")
                  for h in range(2)]
            outtz = [sb.tile([P, S], f16, tag=f"outtz{h}", name=f"outtz{h}")
                     for h in range(2)]
            wosz = [sb.tile([P, D], f16, tag=f"wosz{h}", name=f"wosz{h}")
                    for h in range(2)]

            # zero bands + ones-fill on the (otherwise idle) gpsimd engine;
            # bands first: the first scores matmul needs them, vb is needed
            # slightly later by the first AV matmul
            nc.gpsimd.memset(qtz[0][DK:P, :], 0.0)
            nc.gpsimd.memset(ktz[0][DK:P, :], 0.0)
            nc.gpsimd.memset(qtz[1][0:DK, :], 0.0)
            nc.gpsimd.memset(ktz[1][0:DK, :], 0.0)
            nc.gpsimd.memset(vb[0][:, :], 1.0)
            nc.gpsimd.memset(vb[1][:, :], 1.0)
            nc.gpsimd.memset(outtz[0][DK:P, :], 0.0)
            nc.gpsimd.memset(outtz[1][DK:P, :], 0.0)
            nc.gpsimd.memset(wosz[0][DK:P, :], 0.0)
            nc.gpsimd.memset(wosz[1][DK:P, :], 0.0)

            # ---- phase 1: load x + weights, QKV projections, build V -------
            xts = [sb.tile([P, S], f16, tag=f"xt{dc}", name=f"xt{dc}")
                   for dc in range(NDC)]
            # x first -- it is the startup critical path; quarter-tile
            # descriptors spread across DMA rings with fine-grained deps
            wsb = {}
            for name in ("v", "k", "q"):
                wsb[name] = sb.tile([P, NDC * P], f16, tag=f"w{name}",
                                    name=f"w{name}")
            # weights first (0.4MB, ~1.5us) so the first projection matmul
            # isn't gated behind the 4MB x stream; then x quarter-tiles
            for name, dram in (("v", wvS), ("k", wkS), ("q", wqS)):
                nc.sync.dma_start(wsb[name][:, :], dram[:, :])
            for quart in range(4):
                hs = slice(quart * (S // 4), (quart + 1) * (S // 4))
                for dc in range(NDC):
                    nc.sync.dma_start(xts[dc][:, hs], xT[dc * P:(dc + 1) * P, hs])
            nc.sync.dma_start(wosz[0][0:DK, :], woT0[:, :])
            nc.sync.dma_start(wosz[1][0:DK, :], woT1[:, :])

            vt = sb.tile([P, S], f32, tag="vt")
            ident = sb.tile([P, P], f32, tag="ident")
            make_identity(nc, ident[:, :])

            psn = 0
            for st in range(NST):
                for name in ("v", "k", "q"):
                    w = wsb[name]
                    ps = av_ps(psn, [P, NQ])
                    psn += 1
                    for dc in range(NDC):
                        nc.tensor.matmul(
                            ps[:, :],
                            w[:, dc * P:(dc + 1) * P],
                            xts[dc][:, st * NQ:(st + 1) * NQ],
                            start=(dc == 0),
                            stop=(dc == NDC - 1),
                        )
                    sl = slice(st * NQ, (st + 1) * NQ)
                    if name == "v":
                        nc.vector.tensor_copy(vt[:, sl], ps[:, :])
                        # transpose this V window into per-head V chunks
                        # [kpos, dk] (+ones col at 64) right away
                        for ch in range(4 * st, 4 * st + 4):
                            tp = av_ps(psn, [P, P])
                            psn += 1
                            nc.tensor.transpose(
                                tp[:, :], vt[:, ch * P:(ch + 1) * P], ident[:, :]
                            )
                            c0 = ch * (DK + 1)
                            nc.scalar.copy(vb[0][:, c0:c0 + DK], tp[:, 0:DK])
                            nc.vector.tensor_copy(
                                vb[1][:, c0:c0 + DK], tp[:, DK:P]
                            )
                    elif name == "k":
                        nc.scalar.copy(ktz[0][0:DK, sl], ps[0:DK, :])
                        nc.scalar.copy(ktz[1][DK:P, sl], ps[DK:P, :])
                    else:
                        nc.vector.tensor_copy(qtz[0][0:DK, sl], ps[0:DK, :])
                        nc.vector.tensor_copy(qtz[1][DK:P, sl], ps[DK:P, :])

            # ---- phase 2: flash attention -----------------------------------
            def emit_normalize(qc, av):
                # evacuate av psum fast (frees the bank), then the softmax
                # division off the critical path in SBUF; sub-major so each
                # s-tile's W_O can launch after ITS sub's muls
                for sub in range(2):
                    for h in range(2):
                        a = av[h, sub]
                        raw = sb.tile([DK + 1, NQ], f32, tag=f"raw{h}{sub}",
                                      name=f"raw{h}{sub}", bufs=3)
                        nc.vector.tensor_copy(raw[:, :], a[0:DK + 1, :])
                        dn0 = sb.tile([P, NQ], f32, tag="dn0", bufs=4)
                        nc.sync.dma_start(dn0[0:1, :], raw[DK:DK + 1, :])
                        rc = sb.tile([P, NQ], f32, tag="rc", bufs=4)
                        nc.vector.reciprocal_approx_fast(rc[0:1, :], dn0[0:1, :])
                        rcb = sb.tile([DK, NQ], f32, tag="rcb", bufs=4)
                        nc.gpsimd.partition_broadcast(
                            rcb[:, :], rc[0:1, :], channels=DK
                        )
                        q0 = qc * QCH + sub * NQ
                        nc.vector.tensor_mul(
                            outtz[h][0:DK, q0:q0 + NQ], raw[0:DK, :], rcb[:, :]
                        )
                    emit_wo_st(2 * qc + sub)

            # W_O for one q-chunk's two s-tiles -- emitted right after
            # that chunk's normalize so the projection overlaps the next
            # chunk's attention instead of tailing the kernel
            def emit_wo_st(st):
                nonlocal psn
                if True:
                    for ec in range(NDC):
                        yp = av_ps(psn, [P, NQ])
                        psn += 1
                        for h in range(2):
                            nc.tensor.matmul(
                                yp[:, :],
                                wosz[h][:, ec * P:(ec + 1) * P],
                                outtz[h][:, st * NQ:(st + 1) * NQ],
                                start=(h == 0),
                                stop=(h == 1),
                            )
                        ys = sb.tile([P, NQ], f32, tag="ys", bufs=4)
                        if st >= NST - 2:
                            # kernel tail: no more exp work on the DVE, so
                            # evacuating there halves the final serial chain
                            nc.vector.tensor_copy(ys[:, :], yp[:, :])
                        else:
                            nc.scalar.copy(ys[:, :], yp[:, :])
                        nc.sync.dma_start(
                            yT[ec * P:(ec + 1) * P, st * NQ:(st + 1) * NQ],
                            ys[:, :],
                        )

            pending = None
            av_pending = None

            def emit_av(k, exs):
                c0 = k * (DK + 1)
                for h in range(2):
                    for sub in range(2):
                        nc.tensor.matmul(
                            av[h, sub][0:DK + 1, :],
                            vb[h][:, c0:c0 + DK + 1],
                            exs[h][:, sub * NQ:(sub + 1) * NQ],
                            start=(k == 0),
                            stop=(k == NKC - 1),
                        )

            for qc in range(NQC):
                av = {}
                for h in range(2):
                    for sub in range(2):
                        av[h, sub] = av_ps(psn, [P, NQ])
                        psn += 1
                for k in range(NKC):
                    if k == 3 and pending is not None:
                        emit_normalize(*pending)
                        pending = None
                    # one single-bank psum tile per scores matmul, so each
                    # exp sub-op depends on exactly one matmul and each WAR
                    # on exactly one exp op (no 2-bank tile-level coupling)
                    scps = [[psp.tile([P, NQ], f32, tag=f"sc{h}{sub}",
                                      name=f"sc{h}{sub}")
                             for sub in range(2)] for h in range(2)]
                    for h in range(2):
                        for sub in range(2):
                            q0 = qc * QCH + sub * NQ
                            nc.tensor.matmul(
                                scps[h][sub][:, :],
                                ktz[h][:, k * P:(k + 1) * P],
                                qtz[h][:, q0:q0 + NQ],
                                start=True,
                                stop=True,
                            )
                    # exp: head0 on ACT (true exp), head1 on the DVE as a
                    # Schraudolph bit-trick exp -- int16(A*s+B) whose bit
                    # pattern read as fp16 is exp(s/8) within ~+-3%
                    # (mean-free; the softmax ratio cancels most of it)
                    ex0 = sb.tile([P, QCH], f16, tag="ex0", name="ex0", bufs=6)
                    ex1i = sb.tile([P, QCH], i16, tag="ex1", name="ex1", bufs=6)
                    for sub in range(2):
                        ssl = slice(sub * NQ, (sub + 1) * NQ)
                        nc.scalar.activation(
                            ex0[:, ssl], scps[0][sub][:, :], Act.Exp,
                            scale=0.125
                        )
                        nc.vector.tensor_scalar(
                            ex1i[:, ssl], scps[1][sub][:, :], EXP_A, EXP_B,
                            Alu.mult, Alu.add,
                        )
                    exs = [ex0, ex1i.bitcast(f16)]
                    # defer this chunk's AV by one chunk: when emitted, all
                    # its exp inputs are long done, so the PE never waits on
                    # the chunk's last exp op (pins the period to the matmul
                    # streaming floor)
                    if av_pending is not None:
                        emit_av(*av_pending)
                    av_pending = (k, exs)
                for k2, exs2 in (av_pending,):
                    emit_av(k2, exs2)
                av_pending = None
                pending = (qc, av)
            emit_normalize(*pending)

    nc.compile()
    return nc


def _wsb(w, e0):
    # SBUF weight layout: w_sb[p, dc*P + m] = w[e0 + m, dc*P + p]
    wt = w[e0:e0 + P].T.astype(np.float16)          # [D, P]
    return np.ascontiguousarray(
        np.hstack([wt[d * P:(d + 1) * P, :] for d in range(NDC)])
    )


def kernel(x, wq, wk, wv, wo):
    global LAST_RESULTS
    from concourse.bass_utils import run_bass_kernel_spmd

    if "nc" not in _CACHE:
        _CACHE["nc"] = _build_nc()
    nc = _CACHE["nc"]

    x = np.asarray(x, dtype=np.float32)
    wq = np.asarray(wq, dtype=np.float32)
    wk = np.asarray(wk, dtype=np.float32)
    wv = np.asarray(wv, dtype=np.float32)
    wo = np.asarray(wo, dtype=np.float32)

    in_maps = []
    for c in range(8):
        b, hp = divmod(c, 4)
        e0 = hp * P
        in_maps.append({
            "xT": np.ascontiguousarray(x[b].T.astype(np.float16)),
            "wqS": _wsb(wq, e0),
            "wkS": _wsb(wk, e0),
            "wvS": _wsb(wv, e0),
            "woT0": np.ascontiguousarray(wo[:, e0:e0 + DK].T.astype(np.float16)),
            "woT1": np.ascontiguousarray(wo[:, e0 + DK:e0 + P].T.astype(np.float16)),
        })

    res = run_bass_kernel_spmd(
        nc, in_maps, core_ids=list(range(8)), trace=TRACE, tmpdir=TMPDIR
    )
    LAST_RESULTS = res

    y = np.zeros((B, S, D), dtype=np.float32)
    for c in range(8):
        y[c // 4] += res.results[c]["yT"].T
    return y



# revision 35
# speedup vs baseline: 1.1230x; 1.0702x over previous
# Multi-head self-attention (B=2, S=4096, D=512, H=8) on 8 NeuronCores.
#
# Sharding: core c -> batch b = c//4, head-pair hp = c%4 (heads 2hp, 2hp+1,
# i.e. channels [128*hp, 128*hp+128) of the QKV projection space).
# Host pre-slices/transposes weights + x per core (cast fp16 for the PE);
# device does all matmuls (QKV projections, flash-attention with fused
# softmax, W_O row-slice projection); host sums the 4 per-core W_O partials
# per batch (the "all-reduce") and transposes back.
#
# Per-core device kernel (matmul operands fp16, accumulation fp32 PSUM):
#   qtz_h/ktz_h [128, 4096]: Q^T/K^T per head, dk on a 64-partition band,
#     zero elsewhere -> every attention matmul is full-K (128,128) mode.
#   scoresT[kpos, q] = K Q^T chunkwise -> ACT exp(x/8) straight from PSUM
#   AV with a ones-column appended to V -> denominator for free
#   softmax division off the critical path (DVE recip + gpsimd broadcast).
# All pools stay open for the whole kernel; PSUM slots are shared between
# phases via tags (8 banks total) so phases overlap with per-slot WAR deps
# instead of pool-close barriers.

import numpy as np

B, S, D, H, DK = 2, 4096, 512, 8, 64
# Schraudolph fp16 exp: bits = round(A*s + B) read as fp16 ~= exp(s/8);
# 0.0430357 centers the multiplicative error of the linear-mantissa
# approximation at +-3%.
EXP_A = 1024.0 * 1.4426950408889634 / 8.0
EXP_B = 15360.0 - 1024.0 * 0.04303566
P = 128          # partition tile
NQ = 512         # matmul moving free dim (one fp32 PSUM bank)
QCH = 1024       # q-chunk (2 x NQ) => one [128,1024] exp per kpos-chunk
NKC = S // P     # kpos chunks (32)
NST = S // NQ    # s-tiles of 512 (8)
NDC = D // P     # d chunks (4)
NQC = S // QCH   # q chunks (4)

TRACE = False            # test.py sets True to get exec_time_ns + perfetto
TMPDIR = None            # optional trace output dir
LAST_RESULTS = None      # BassKernelResults of the last run (for test.py)

_CACHE = {}


def _build_nc():
    import concourse.bass as bass  # noqa: F401
    import concourse.mybir as mybir
    import concourse.tile as tile
    from concourse import bacc

    f32 = mybir.dt.float32
    f16 = mybir.dt.float16
    i16 = mybir.dt.int16
    Act = mybir.ActivationFunctionType
    Alu = mybir.AluOpType

    nc = bacc.Bacc("TRN2", target_bir_lowering=False, debug=False, num_devices=8)

    xT = nc.dram_tensor("xT", [D, S], f16, kind="ExternalInput")
    # weight slices pre-arranged by the host in the exact SBUF layout
    # ([P, D] row-slice of the torch weight) -> one contiguous DMA each
    wqS = nc.dram_tensor("wqS", [P, D], f16, kind="ExternalInput")
    wkS = nc.dram_tensor("wkS", [P, D], f16, kind="ExternalInput")
    wvS = nc.dram_tensor("wvS", [P, D], f16, kind="ExternalInput")
    woT0 = nc.dram_tensor("woT0", [DK, D], f16, kind="ExternalInput")
    woT1 = nc.dram_tensor("woT1", [DK, D], f16, kind="ExternalInput")
    yT = nc.dram_tensor("yT", [D, S], f32, kind="ExternalOutput")

    with tile.TileContext(nc) as tc:
        with (
            tc.tile_pool(name="sb", bufs=1) as sb,
            tc.tile_pool(name="ps", bufs=1, space="PSUM") as psp,
        ):
            # PSUM budget (8 banks total, slots shared across phases by tag):
            #   sc0, sc1: [128,1024] -> 2 banks each (scores / exp staging)
            #   av00..av11: [128,512] -> 1 bank each (AV accum; also used by
            #   the QKV-projection psum tiles and the W_O psum tiles)
            def av_ps(i, shape):
                return psp.tile(shape, f32, tag=f"av{i % 4}", name=f"avps{i}")

            # ---- persistent operand tiles -----------------------------------
            qtz = [sb.tile([P, S], f16, tag=f"qtz{h}", name=f"qtz{h}")
                   for h in range(2)]
            ktz = [sb.tile([P, S], f16, tag=f"ktz{h}", name=f"ktz{h}")
                   for h in range(2)]
            vb = [sb.tile([P, NKC * (DK + 1)], f16, tag=f"vb{h}", name=f"vb{h}")
                  for h in range(2)]
            outtz = [sb.tile([P, S], f16, tag=f"outtz{h}", name=f"outtz{h}")
                     for h in range(2)]
            wosz = [sb.tile([P, D], f16, tag=f"wosz{h}", name=f"wosz{h}")
                    for h in range(2)]

            # zero bands + ones-fill on the (otherwise idle) gpsimd engine;
            # bands first: the first scores matmul needs them, vb is needed
            # slightly later by the first AV matmul
            nc.gpsimd.memset(qtz[0][DK:P, :], 0.0)
            nc.gpsimd.memset(ktz[0][DK:P, :], 0.0)
            nc.gpsimd.memset(qtz[1][0:DK, :], 0.0)
            nc.gpsimd.memset(ktz[1][0:DK, :], 0.0)
            nc.gpsimd.memset(vb[0][:, :], 1.0)
            nc.gpsimd.memset(vb[1][:, :], 1.0)
            nc.gpsimd.memset(outtz[0][DK:P, :], 0.0)
            nc.gpsimd.memset(outtz[1][DK:P, :], 0.0)
            nc.gpsimd.memset(wosz[0][DK:P, :], 0.0)
            nc.gpsimd.memset(wosz[1][DK:P, :], 0.0)

            # ---- phase 1: load x + weights, QKV projections, build V -------
            xts = [sb.tile([P, S], f16, tag=f"xt{dc}", name=f"xt{dc}")
                   for dc in range(NDC)]
            # x first -- it is the startup critical path; quarter-tile
            # descriptors spread across DMA rings with fine-grained deps
            wsb = {}
            for name in ("v", "k", "q"):
                wsb[name] = sb.tile([P, NDC * P], f16, tag=f"w{name}",
                                    name=f"w{name}")
            # weights first (0.4MB, ~1.5us) so the first projection matmul
            # isn't gated behind the 4MB x stream; then x quarter-tiles
            for name, dram in (("v", wvS), ("k", wkS), ("q", wqS)):
                nc.sync.dma_start(wsb[name][:, :], dram[:, :])
            for quart in range(4):
                hs = slice(quart * (S // 4), (quart + 1) * (S // 4))
                for dc in range(NDC):
                    nc.sync.dma_start(xts[dc][:, hs], xT[dc * P:(dc + 1) * P, hs])
            nc.sync.dma_start(wosz[0][0:DK, :], woT0[:, :])
            nc.sync.dma_start(wosz[1][0:DK, :], woT1[:, :])

            psn = 0
            for st in range(NST):
                for name in ("v", "k", "q"):
                    w = wsb[name]
                    if name == "v":
                        # V projected directly into [kpos, dk] layout (x
                        # chunk stationary, wv moving): no PE transposes,
                        # no fp32 staging
                        for ch in range(4 * st, 4 * st + 4):
                            vps = av_ps(psn, [P, P])
                            psn += 1
                            for dc in range(NDC):
                                nc.tensor.matmul(
                                    vps[:, :],
                                    xts[dc][:, ch * P:(ch + 1) * P],
                                    w[:, dc * P:(dc + 1) * P],
                                    start=(dc == 0),
                                    stop=(dc == NDC - 1),
                                )
                            c0 = ch * (DK + 1)
                            nc.scalar.copy(vb[0][:, c0:c0 + DK], vps[:, 0:DK])
                            nc.vector.tensor_copy(
                                vb[1][:, c0:c0 + DK], vps[:, DK:P]
                            )
                        continue
                    ps = av_ps(psn, [P, NQ])
                    psn += 1
                    for dc in range(NDC):
                        nc.tensor.matmul(
                            ps[:, :],
                            w[:, dc * P:(dc + 1) * P],
                            xts[dc][:, st * NQ:(st + 1) * NQ],
                            start=(dc == 0),
                            stop=(dc == NDC - 1),
                        )
                    sl = slice(st * NQ, (st + 1) * NQ)
                    if name == "k":
                        nc.scalar.copy(ktz[0][0:DK, sl], ps[0:DK, :])
                        nc.scalar.copy(ktz[1][DK:P, sl], ps[DK:P, :])
                    else:
                        nc.vector.tensor_copy(qtz[0][0:DK, sl], ps[0:DK, :])
                        nc.vector.tensor_copy(qtz[1][DK:P, sl], ps[DK:P, :])

            # ---- phase 2: flash attention -----------------------------------
            def emit_normalize(qc, av):
                # evacuate av psum fast (frees the bank), then the softmax
                # division off the critical path in SBUF; sub-major so each
                # s-tile's W_O can launch after ITS sub's muls
                for sub in range(2):
                    for h in range(2):
                        a = av[h, sub]
                        raw = sb.tile([DK + 1, NQ], f32, tag=f"raw{h}{sub}",
                                      name=f"raw{h}{sub}", bufs=3)
                        nc.vector.tensor_copy(raw[:, :], a[0:DK + 1, :])
                        dn0 = sb.tile([P, NQ], f32, tag="dn0", bufs=4)
                        nc.sync.dma_start(dn0[0:1, :], raw[DK:DK + 1, :])
                        rc = sb.tile([P, NQ], f32, tag="rc", bufs=4)
                        nc.vector.reciprocal_approx_fast(rc[0:1, :], dn0[0:1, :])
                        rcb = sb.tile([DK, NQ], f32, tag="rcb", bufs=4)
                        nc.gpsimd.partition_broadcast(
                            rcb[:, :], rc[0:1, :], channels=DK
                        )
                        q0 = qc * QCH + sub * NQ
                        nc.vector.tensor_mul(
                            outtz[h][0:DK, q0:q0 + NQ], raw[0:DK, :], rcb[:, :]
                        )
                    emit_wo_st(2 * qc + sub)

            # W_O for one q-chunk's two s-tiles -- emitted right after
            # that chunk's normalize so the projection overlaps the next
            # chunk's attention instead of tailing the kernel
            def emit_wo_st(st):
                nonlocal psn
                if True:
                    for ec in range(NDC):
                        yp = av_ps(psn, [P, NQ])
                        psn += 1
                        for h in range(2):
                            nc.tensor.matmul(
                                yp[:, :],
                                wosz[h][:, ec * P:(ec + 1) * P],
                                outtz[h][:, st * NQ:(st + 1) * NQ],
                                start=(h == 0),
                                stop=(h == 1),
                            )
                        ys = sb.tile([P, NQ], f32, tag="ys", bufs=4)
                        if st >= NST - 2:
                            # kernel tail: no more exp work on the DVE, so
                            # evacuating there halves the final serial chain
                            nc.vector.tensor_copy(ys[:, :], yp[:, :])
                        else:
                            nc.scalar.copy(ys[:, :], yp[:, :])
                        nc.sync.dma_start(
                            yT[ec * P:(ec + 1) * P, st * NQ:(st + 1) * NQ],
                            ys[:, :],
                        )

            pending = None
            av_pending = None

            def emit_av(k, exs):
                c0 = k * (DK + 1)
                for h in range(2):
                    for sub in range(2):
                        nc.tensor.matmul(
                            av[h, sub][0:DK + 1, :],
                            vb[h][:, c0:c0 + DK + 1],
                            exs[h][:, sub * NQ:(sub + 1) * NQ],
                            start=(k == 0),
                            stop=(k == NKC - 1),
                        )

            for qc in range(NQC):
                av = {}
                for h in range(2):
                    for sub in range(2):
                        av[h, sub] = av_ps(psn, [P, NQ])
                        psn += 1
                for k in range(NKC):
                    if k == 3 and pending is not None:
                        emit_normalize(*pending)
                        pending = None
                    # one single-bank psum tile per scores matmul, so each
                    # exp sub-op depends on exactly one matmul and each WAR
                    # on exactly one exp op (no 2-bank tile-level coupling)
                    scps = [[psp.tile([P, NQ], f32, tag=f"sc{h}{sub}",
                                      name=f"sc{h}{sub}")
                             for sub in range(2)] for h in range(2)]
                    for h in range(2):
                        for sub in range(2):
                            q0 = qc * QCH + sub * NQ
                            nc.tensor.matmul(
                                scps[h][sub][:, :],
                                ktz[h][:, k * P:(k + 1) * P],
                                qtz[h][:, q0:q0 + NQ],
                                start=True,
                                stop=True,
                            )
                    # exp: head0 on ACT (true exp), head1 on the DVE as a
                    # Schraudolph bit-trick exp -- int16(A*s+B) whose bit
                    # pattern read as fp16 is exp(s/8) within ~+-3%
                    # (mean-free; the softmax ratio cancels most of it)
                    ex0 = sb.tile([P, QCH], f16, tag="ex0", name="ex0", bufs=6)
                    ex1i = sb.tile([P, QCH], i16, tag="ex1", name="ex1", bufs=6)
                    for sub in range(2):
                        ssl = slice(sub * NQ, (sub + 1) * NQ)
                        nc.scalar.activation(
                            ex0[:, ssl], scps[0][sub][:, :], Act.Exp,
                            scale=0.125
                        )
                        nc.vector.tensor_scalar(
                            ex1i[:, ssl], scps[1][sub][:, :], EXP_A, EXP_B,
                            Alu.mult, Alu.add,
                        )
                    exs = [ex0, ex1i.bitcast(f16)]
                    # defer this chunk's AV by one chunk: when emitted, all
                    # its exp inputs are long done, so the PE never waits on
                    # the chunk's last exp op (pins the period to the matmul
                    # streaming floor)
                    if av_pending is not None:
                        emit_av(*av_pending)
                    av_pending = (k, exs)
                for k2, exs2 in (av_pending,):
                    emit_av(k2, exs2)
                av_pending = None
                pending = (qc, av)
            emit_normalize(*pending)

    nc.compile()
    return nc


def _wsb(w, e0):
    # SBUF weight layout: w_sb[p, dc*P + m] = w[e0 + m, dc*P + p]
    wt = w[e0:e0 + P].T.astype(np.float16)          # [D, P]
    return np.ascontiguousarray(
        np.hstack([wt[d * P:(d + 1) * P, :] for d in range(NDC)])
    )


def kernel(x, wq, wk, wv, wo):
    global LAST_RESULTS
    from concourse.bass_utils import run_bass_kernel_spmd

    if "nc" not in _CACHE:
        _CACHE["nc"] = _build_nc()
    nc = _CACHE["nc"]

    x = np.asarray(x, dtype=np.float32)
    wq = np.asarray(wq, dtype=np.float32)
    wk = np.asarray(wk, dtype=np.float32)
    wv = np.asarray(wv, dtype=np.float32)
    wo = np.asarray(wo, dtype=np.float32)

    in_maps = []
    for c in range(8):
        b, hp = divmod(c, 4)
        e0 = hp * P
        in_maps.append({
            "xT": np.ascontiguousarray(x[b].T.astype(np.float16)),
            "wqS": _wsb(wq, e0),
            "wkS": _wsb(wk, e0),
            "wvS": _wsb(wv, e0),
            "woT0": np.ascontiguousarray(wo[:, e0:e0 + DK].T.astype(np.float16)),
            "woT1": np.ascontiguousarray(wo[:, e0 + DK:e0 + P].T.astype(np.float16)),
        })

    res = run_bass_kernel_spmd(
        nc, in_maps, core_ids=list(range(8)), trace=TRACE, tmpdir=TMPDIR
    )
    LAST_RESULTS = res

    y = np.zeros((B, S, D), dtype=np.float32)
    for c in range(8):
        y[c // 4] += res.results[c]["yT"].T
    return y

